# revision 1
# baseline (speedup 1.0000x reference)
"""Trainium2 Bass kernel for nn_CausalSelfAttention_39213051412899.

Sliding-window causal GQA attention with value-embedding gate.
Sharding: 8 cores = batch(2) x kv-group(4).  Each core computes its batch's
4 query heads / 1 kv head and a row-parallel partial of the output
projection; the host sums the 4 partials per batch.

Self-contained: only needs numpy + the concourse tree staged in the
container at /opt/trn_rl_repo (environment, not problem files).
"""

import os
import sys

import numpy as np

try:
    import concourse.bass as bass  # noqa: F401
except ImportError:  # pragma: no cover
    sys.path.insert(0, "/opt/trn_rl_repo")

import concourse.bass as bass
import concourse.tile as tile
from concourse import bacc
from concourse import mybir
from concourse.bass_utils import run_bass_kernel_spmd

F32 = mybir.dt.float32
F32R = mybir.dt.float32r
AF = mybir.ActivationFunctionType
ALU = mybir.AluOpType

B, S, E = 2, 2048, 1024
H, KV, D = 16, 4, 64
G = H // KV          # 4 q heads per kv head (per core)
GC = 128             # gate channels
EPS = 1.1920929e-07
T = S // 128         # 16 s-tiles
QKVW = G * D + D + D + 2   # 386 packed cols (q|k|v|gate|pad) - fp32r needs even N
GATE_COL = G * D + 2 * D   # 384
NCORES = 8
HT = T // 2          # tiles per half

_cache = {}
last_results = None   # test harness reads exec_time_ns off this


def _block_range(ti, tj, lo_delta, hi_delta):
    """Active query-tile range for key-tile tj (None if empty)."""
    lo = max(ti[0], tj - hi_delta)
    hi = min(ti[1], tj - lo_delta)
    if lo > hi:
        return None
    return lo, hi



def _mask_table(wl, wr_eff):
    """Distinct additive mask tiles needed, keyed (kind, base) -> index."""
    lo_delta = -((127 + wl) // 128)
    hi_delta = (127 + wr_eff) // 128
    keys = {}
    for dt_ in range(lo_delta, hi_delta + 1):   # dt_ = tj - tb
        bw = wl + 128 * dt_
        if -127 <= bw < 127:
            keys.setdefault(("w", bw), len(keys))
        bc = wr_eff - 128 * dt_
        if -127 <= bc < 127:
            keys.setdefault(("c", bc), len(keys))
    return keys


def _mask_tiles(wl, wr_eff):
    keys = _mask_table(wl, wr_eff)
    n = max(1, len(keys))
    m = np.zeros((128, n * 128), np.float32)
    rj = np.arange(128)[:, None]
    ri = np.arange(128)[None, :]
    for (kind, base), i in keys.items():
        if kind == "w":
            bad = (rj - ri + base) < 0
        else:
            bad = (ri - rj + base) < 0
        m[:, i * 128:(i + 1) * 128] = np.where(bad, -1e30, 0.0)
    return m


def _build(wl, wr):
    wr_eff = min(int(wr), 0)
    wl = int(wl)
    lo_delta = -((127 + wl) // 128)    # tj - ti >= lo_delta
    hi_delta = (127 + wr_eff) // 128   # tj - ti <= hi_delta  (0 when wr>=0)

    nc = bacc.Bacc(None, target_bir_lowering=False)
    d_xT = nc.declare_dram_parameter("xT", [E, S], F32R, isOutput=False)
    d_wqkv = nc.declare_dram_parameter("wqkv", [E, QKVW], F32R, isOutput=False)
    d_cos4 = nc.declare_dram_parameter("cos4", [128, S], F32, isOutput=False)
    d_sin4 = nc.declare_dram_parameter("sin4", [128, S], F32, isOutput=False)
    d_ve2 = nc.declare_dram_parameter("ve2", [128, T * D], F32, isOutput=False)
    d_wproj = nc.declare_dram_parameter("wproj", [G * D, E], F32R, isOutput=False)
    d_ident = nc.declare_dram_parameter("ident", [128, 128], F32R, isOutput=False)
    d_zero = nc.declare_dram_parameter("zero512", [128, 512], F32R, isOutput=False)
    d_ones = nc.declare_dram_parameter("ones16", [128, T], F32R, isOutput=False)
    mask_idx = _mask_table(wl, wr_eff)
    nmask = max(1, len(mask_idx))
    d_masks = nc.declare_dram_parameter("masks", [128, nmask * 128], F32,
                                        isOutput=False)
    d_out = nc.declare_dram_parameter("outp", [S, E], F32, isOutput=True)

    with tile.TileContext(nc) as tc:
        with tc.tile_pool(name="persist", bufs=1) as persist:
            qT = [persist.tile([128, S], F32R, tag=f"qT{i}", name=f"qT{i}") for i in range(2)]  # 2 heads/tile
            kT = persist.tile([128, S], F32R)                      # k duplicated
            vaug = persist.tile([128, T, D + 1], F32R)             # v | ones col
            yT = [persist.tile([128, S], F32R, tag=f"yT{i}", name=f"yT{i}") for i in range(2)]
            ident = persist.tile([128, 128], F32R)
            sig = persist.tile([128, T], F32)
            stat = persist.tile([128, 5 * T], F32)  # q ssum (64) | k ssum (16)
            rs = persist.tile([128, 5 * T], F32)
            Lt = persist.tile([16, 1024], F32)
            masks = persist.tile([128, nmask * 128], F32)
            epst = persist.tile([128, 1], F32)
            Linv = persist.tile([16, 1024], F32)

            nc.sync.dma_start(ident[:], d_ident[:, :])
            nc.sync.dma_start(masks[:], d_masks[:, :])
            nc.vector.memset(epst[:], EPS)
            nc.sync.dma_start(vaug[:, :, D], d_ones[:, :])

            # ---------------- phase 1: projections (both halves) -----------
            with (
                tc.tile_pool(name="ph1", bufs=1) as ph1,
                tc.tile_pool(name="ph1x", bufs=2) as ph1x,
                tc.tile_pool(name="ph1s", bufs=2) as ph1s,
                tc.tile_pool(name="ph1t", bufs=1) as ph1t,
                tc.tile_pool(name="pq", bufs=3, space="PSUM") as pq,
                tc.tile_pool(name="ptr", bufs=3, space="PSUM") as ptr,
            ):
                wq_s = ph1.tile([128, 8, QKVW], F32R)
                cos4 = ph1.tile([128, S], F32)
                sin4 = ph1.tile([128, S], F32)
                ve2 = ph1.tile([128, T * D], F32)
                for c in range(8):
                    nc.sync.dma_start(wq_s[:, c, :],
                                      d_wqkv[c * 128:(c + 1) * 128, :])
                nc.sync.dma_start(cos4[:], d_cos4[:, :])
                nc.sync.dma_start(sin4[:], d_sin4[:, :])
                nc.sync.dma_start(ve2[:], d_ve2[:, :])

                qkvs = []
                for hf in range(2):
                    xts = ph1x.tile([128, 8, HT * 128], F32R, tag="xts")
                    for c in range(8):
                        nc.sync.dma_start(
                            xts[:, c, :],
                            d_xT[c * 128:(c + 1) * 128,
                                 hf * HT * 128:(hf + 1) * HT * 128])
                    qkv = ph1s.tile([128, HT, QKVW], F32R, tag="qkv")
                    for t in range(HT):
                        ps = pq.tile([128, QKVW], F32)
                        for c in range(8):
                            nc.tensor.matmul(
                                ps[:], (xts[:, c, t * 128:(t + 1) * 128]),
                                (wq_s[:, c, :]),
                                start=(c == 0), stop=(c == 7))
                        nc.scalar.copy(qkv[:, t, :], ps[:])
                    qkvs.append(qkv)

                # ---------- phase 2: gate/rope/rms/transposes per half -----
                for hf in range(2):
                    t0 = hf * HT
                    qkv = qkvs[hf]
                    nc.scalar.activation(
                        sig[:, t0:t0 + HT],
                        qkv[:, :, GATE_COL:GATE_COL + 1].rearrange("p t o -> p (t o)"),
                        AF.Sigmoid)

                    qv = qkv[:, :, 0:G * D].rearrange("p t (h d) -> p t h d", h=G)
                    kv_ = qkv[:, :, G * D:G * D + D].rearrange("p t (o d) -> p t o d", o=1)
                    cosq = cos4[:, t0 * 128:(t0 + HT) * 128].rearrange(
                        "p (t h f) -> p t h f", h=G, f=32)
                    sinq = sin4[:, t0 * 128:(t0 + HT) * 128].rearrange(
                        "p (t h f) -> p t h f", h=G, f=32)
                    rq = ph1t.tile([128, HT, G, D], F32, tag="rq")
                    rk = ph1t.tile([128, HT, 1, D], F32, tag="rk")
                    tmp = ph1t.tile([128, HT, G, 32], F32, tag="tmp")
                    tmpk = ph1t.tile([128, HT, 1, 32], F32, tag="tmpk")
                    for (src, dst, cs, sn, tm) in (
                            (qv, rq, cosq, sinq, tmp),
                            (kv_, rk, cosq[:, :, 0:1, :], sinq[:, :, 0:1, :],
                             tmpk)):
                        x1 = src[:, :, :, 0:32]
                        x2 = src[:, :, :, 32:64]
                        nc.vector.tensor_mul(dst[:, :, :, 0:32], x1, cs)
                        nc.vector.tensor_mul(tm[:], x2, sn)
                        nc.vector.tensor_add(
                            dst[:, :, :, 0:32], dst[:, :, :, 0:32], tm[:])
                        nc.vector.tensor_mul(dst[:, :, :, 32:64], x2, cs)
                        nc.vector.tensor_mul(tm[:], x1, sn)
                        nc.vector.tensor_sub(
                            dst[:, :, :, 32:64], dst[:, :, :, 32:64], tm[:])

                    # rms stats: square into dead qkv q/k region, then reduce
                    nc.vector.tensor_mul(qv, rq[:], rq[:])
                    nc.vector.tensor_reduce(
                        op=ALU.add,
                        out=stat[:, t0 * 4:(t0 + HT) * 4].rearrange(
                            "p (t h) -> p t h", h=G),
                        in_=qv, axis=mybir.AxisListType.X)
                    nc.vector.tensor_mul(kv_, rk[:], rk[:])
                    nc.vector.tensor_reduce(
                        op=ALU.add,
                        out=stat[:, 64 + t0:64 + t0 + HT].rearrange(
                            "p (t h) -> p t h", h=1),
                        in_=kv_, axis=mybir.AxisListType.X)
                    for sl in (slice(t0 * 4, (t0 + HT) * 4),
                               slice(64 + t0, 64 + t0 + HT)):
                        nc.scalar.activation(rs[:, sl], stat[:, sl], AF.Sqrt,
                                             bias=epst[:], scale=1.0 / D)
                        nc.vector.reciprocal(rs[:, sl], rs[:, sl])

                    # normalize back into qkv (per-partition scale on ACT)
                    for t in range(HT):
                        tt = t0 + t
                        for h in range(G):
                            nc.vector.tensor_scalar_mul(
                                qkv[:, t, h * D:(h + 1) * D], rq[:, t, h, :],
                                rs[:, tt * 4 + h:tt * 4 + h + 1])
                        nc.vector.tensor_scalar_mul(
                            qkv[:, t, G * D:G * D + D], rk[:, t, 0, :],
                            rs[:, 64 + tt:64 + tt + 1])

                    # v + sig * ve2  (ve2 pre-scaled by 2 on host)
                    tmpv = ph1t.tile([128, HT, D], F32, tag="tmpv")
                    for t in range(HT):
                        tt = t0 + t
                        nc.scalar.activation(
                            tmpv[:, t, :], ve2[:, tt * D:(tt + 1) * D],
                            AF.Copy, scale=sig[:, tt:tt + 1])
                        nc.vector.tensor_add(
                            vaug[:, tt, 0:D],
                            qkv[:, t, G * D + D:G * D + 2 * D], tmpv[:, t, :])

                    # transposes -> qT (2-head packed), kT (k duplicated)
                    for t in range(HT):
                        tt = t0 + t
                        for bk in range(2):
                            tp = ptr.tile([128, 128], F32R, tag="tp")
                            nc.tensor.transpose(
                                (tp[:]),
                                (qkv[:, t, bk * 128:(bk + 1) * 128]),
                                (ident[:]))
                            nc.vector.tensor_copy(
                                qT[bk][:, tt * 128:(tt + 1) * 128], tp[:])
                        tp = ptr.tile([128, 128], F32R, tag="tp")
                        nc.tensor.transpose(
                            (tp[0:64, :]), (qkv[:, t, G * D:G * D + D]),
                            (ident[:]))
                        nc.vector.tensor_copy(
                            kT[0:64, tt * 128:(tt + 1) * 128], tp[0:64, :])
                    # duplicate k rows into partitions 64-127 (for head-odd
                    # base-partition alignment) once this half is transposed
                    nc.sync.dma_start(
                        kT[64:128, t0 * 128:(t0 + HT) * 128],
                        kT[0:64, t0 * 128:(t0 + HT) * 128])

            # ---------------- phase 3: attention ---------------------------
            # chunk size: widest for which every chunk has a full-span tj
            def _has_full(cs_tiles):
                for C in range(T // cs_tiles):
                    ti_ = (cs_tiles * C, cs_tiles * C + cs_tiles - 1)
                    if not any(_block_range(ti_, tj, lo_delta, hi_delta) == ti_
                               for tj in range(T)):
                        return False
                return True
            CST = 8 if _has_full(8) else 4       # chunk tiles
            CS = CST * 128
            NCH = T // CST
            pad_mode = not _has_full(CST)

            with (
                tc.tile_pool(name="att", bufs=8) as att,
                tc.tile_pool(name="ytu", bufs=G * NCH) as pytu,
                tc.tile_pool(name="lbp", bufs=3) as plb,
                tc.tile_pool(name="ytn", bufs=2) as pytn,
                tc.tile_pool(name="psc", bufs=2, space="PSUM") as psc,
                tc.tile_pool(name="ppv", bufs=2, space="PSUM") as ppv,
                tc.tile_pool(name="dsc", bufs=1, space="DRAM") as dsc,
            ):
                d_linv = dsc.tile([G * NCH, CS], F32)
                ytus = {}

                def chunk_steps(h, C):
                    rh = slice((h % 2) * 64, (h % 2) * 64 + 64)
                    qTh = qT[h // 2]
                    c0 = CST * C
                    ti = (c0, c0 + CST - 1)
                    tjs = [tj for tj in
                           range(max(0, c0 + lo_delta),
                                 min(T - 1, c0 + CST - 1 + hi_delta) + 1)
                           if _block_range(ti, tj, lo_delta, hi_delta)]
                    full = [tj for tj in tjs
                            if _block_range(ti, tj, lo_delta, hi_delta) == ti]
                    if pad_mode:
                        order = tjs
                    else:
                        ftj = full[-1]
                        order = [ftj] + [tj for tj in tjs if tj != ftj]

                    yTa = ppv.tile([65, CS], F32, tag="yTa", name="yTa")
                    half_started = [False] * (CST // 4)
                    half_last = {}
                    for i, tj in enumerate(order):
                        lo_, hi_ = _block_range(ti, tj, lo_delta, hi_delta)
                        o_, n_ = ((0, CS) if (pad_mode and i == 0) else
                                  ((lo_ - c0) * 128, (hi_ - lo_ + 1) * 128))
                        for hx in range(CST // 4):
                            if o_ < (hx + 1) * 512 and o_ + n_ > hx * 512:
                                half_last[hx] = i
                    pend = []

                    def emit_pv(rec):
                        i, tj, pt, off, n = rec
                        for hx in range(CST // 4):
                            h0 = hx * 512
                            s0 = max(off, h0)
                            s1 = min(off + n, h0 + 512)
                            if s0 >= s1:
                                continue
                            first = not half_started[hx]
                            half_started[hx] = True
                            nc.tensor.matmul(
                                yTa[:, s0:s1], vaug[:, tj, :],
                                pt[:, s0:s1],
                                start=first, stop=(half_last[hx] == i))

                    for i, tj in enumerate(order):
                        alo, ahi = _block_range(ti, tj, lo_delta, hi_delta)
                        aoff = (alo - c0) * 128
                        an = (ahi - alo + 1) * 128
                        if pad_mode and i == 0:
                            off, n = 0, CS
                        else:
                            off, n = aoff, an
                        sc = psc.tile([128, CS], F32, tag="sc", name="sc")
                        pt = att.tile([128, CS], F32R, tag="pt", name="pt")
                        p0 = aoff
                        while p0 < aoff + an:
                            p1 = min((p0 // 512 + 1) * 512, aoff + an)
                            nc.tensor.matmul(
                                sc[:, p0:p1],
                                kT[rh, tj * 128:(tj + 1) * 128],
                                qTh[rh, C * CS + p0:C * CS + p1],
                                start=True, stop=True)
                            p0 = p1
                        for tb in range(alo, ahi + 1):
                            bo = (tb - c0) * 128
                            for kind, base in (("w", wl - 128 * (tb - tj)),
                                               ("c", wr_eff + 128 * (tb - tj))):
                                if -127 <= base < 127:
                                    mi = mask_idx[(kind, base)]
                                    nc.vector.tensor_add(
                                        sc[:, bo:bo + 128],
                                        sc[:, bo:bo + 128],
                                        masks[:, mi * 128:(mi + 1) * 128])
                        if pad_mode:
                            nc.sync.dma_start(
                                pt[:], d_zero[:, :].to_broadcast([128, CS]))
                        nc.scalar.activation(
                            pt[:, aoff:aoff + an], sc[:, aoff:aoff + an],
                            AF.Exp, scale=float(D) ** -0.5)
                        pend.append((i, tj, pt, off, n))
                        if len(pend) > 1:
                            emit_pv(pend.pop(0))
                        yield
                    emit_pv(pend.pop(0))

                    ytu = pytu.tile([65, CS], F32, tag="ytu", name="ytu")
                    nc.vector.tensor_copy(ytu[:], yTa[:])
                    rr = h * NCH + C
                    nc.sync.dma_start(Lt[rr:rr + 1, 0:CS], ytu[64:65, :])
                    ytus[(h, C)] = ytu
                    yield

                # drain pairs of equal-length streams interleaved so PE
                # always has an independent chunk to work on
                pairs = []
                for C in range(NCH):
                    for h in range(0, G, 2):
                        pairs.append((chunk_steps(h, C),
                                      chunk_steps(h + 1, C)))
                for gpair in pairs:
                    active = list(gpair)
                    while active:
                        for g in list(active):
                            try:
                                next(g)
                            except StopIteration:
                                active.remove(g)

                nc.vector.reciprocal(Linv[0:G * NCH, 0:CS], Lt[0:G * NCH, 0:CS])
                nc.sync.dma_start(d_linv[:], Linv[0:G * NCH, 0:CS])

                for h in range(G):
                    for C in range(NCH):
                        rr = h * NCH + C
                        ytu = ytus[(h, C)]
                        lb = plb.tile([64, CS], F32)
                        nc.sync.dma_start(
                            lb[:], d_linv[rr:rr + 1, :].to_broadcast([64, CS]))
                        ccols = slice(C * CS, (C + 1) * CS)
                        if h % 2 == 0:
                            nc.vector.tensor_mul(
                                yT[h // 2][0:64, ccols], ytu[0:64, :], lb[:])
                        else:
                            ytn = pytn.tile([64, CS], F32R)
                            nc.vector.tensor_mul(ytn[:], ytu[0:64, :], lb[:])
                            nc.sync.dma_start(yT[h // 2][64:128, ccols], ytn[:])

            # ---------------- phase 4: output projection -------------------
            with (
                tc.tile_pool(name="wp", bufs=1) as pwp,
                tc.tile_pool(name="ob", bufs=4) as pob,
                tc.tile_pool(name="po", bufs=4, space="PSUM") as ppo,
            ):
                wp_s = pwp.tile([128, 2, E], F32R)
                for kc in range(2):
                    nc.sync.dma_start(wp_s[:, kc, :],
                                      d_wproj[kc * 128:(kc + 1) * 128, :])
                for t in range(T):
                    for nh in range(2):
                        po = ppo.tile([128, 512], F32)
                        nc.tensor.matmul(
                            po[:], (yT[0][:, t * 128:(t + 1) * 128]),
                            (wp_s[:, 0, nh * 512:(nh + 1) * 512]),
                            start=True, stop=False)
                        nc.tensor.matmul(
                            po[:], (yT[1][:, t * 128:(t + 1) * 128]),
                            (wp_s[:, 1, nh * 512:(nh + 1) * 512]),
                            start=False, stop=True)
                        ob = pob.tile([128, 512], F32)
                        nc.scalar.copy(ob[:], po[:])
                        nc.sync.dma_start(
                            d_out[t * 128:(t + 1) * 128,
                                  nh * 512:(nh + 1) * 512], ob[:])
    nc.compile()
    return nc


def _prep_inputs(x, ve, cos, sin, Wq, Wk, Wv, Wproj, Wgate):
    cosn = np.asarray(cos, np.float32).reshape(S, 32)
    sinn = np.asarray(sin, np.float32).reshape(S, 32)
    cos4 = np.empty((128, S), np.float32)
    sin4 = np.empty((128, S), np.float32)
    for t in range(T):
        cos4[:, t * 128:(t + 1) * 128] = np.tile(
            cosn[t * 128:(t + 1) * 128], (1, 4))
        sin4[:, t * 128:(t + 1) * 128] = np.tile(
            sinn[t * 128:(t + 1) * 128], (1, 4))

    Wq = np.asarray(Wq, np.float32)
    Wk = np.asarray(Wk, np.float32)
    Wv = np.asarray(Wv, np.float32)
    Wproj = np.asarray(Wproj, np.float32)
    Wgate = np.asarray(Wgate, np.float32)
    maps = []
    wl_ = int(getattr(_prep_inputs, '_wl', 1024))
    wr_ = min(int(getattr(_prep_inputs, '_wr', 0)), 0)
    maskt = _mask_tiles(wl_, wr_)
    for core in range(NCORES):
        b, g = core // 4, core % 4
        xT = np.ascontiguousarray(np.asarray(x[b], np.float32).T)
        wg = np.zeros((E, 1), np.float32)
        wg[:GC, 0] = Wgate[:, g]
        wqkv = np.ascontiguousarray(np.concatenate([
            Wq[:, g * G * D:(g + 1) * G * D],
            Wk[:, g * D:(g + 1) * D],
            Wv[:, g * D:(g + 1) * D],
            wg, np.zeros((E, 1), np.float32)], axis=1))
        veg = 2.0 * np.asarray(ve[b][:, g * D:(g + 1) * D], np.float32)
        ve2 = np.ascontiguousarray(
            veg.reshape(T, 128, D).transpose(1, 0, 2).reshape(128, T * D))
        wproj = np.ascontiguousarray(Wproj[g * G * D:(g + 1) * G * D, :])
        maps.append({"xT": xT, "wqkv": wqkv, "cos4": cos4, "sin4": sin4,
                     "ve2": ve2, "wproj": wproj,
                     "ident": np.eye(128, dtype=np.float32),
                     "ones16": np.ones((128, T), np.float32),
                     "masks": maskt,
                     "zero512": np.zeros((128, 512), np.float32)})
    return maps


def kernel(x, ve, cos, sin, Wq, Wk, Wv, Wproj, Wgate,
           window_left, window_right):
    global last_results
    wl, wr = int(window_left), int(window_right)
    key = (wl, wr)
    if key not in _cache:
        _cache[key] = _build(wl, wr)
    nc = _cache[key]
    _prep_inputs._wl, _prep_inputs._wr = wl, wr
    maps = _prep_inputs(x, ve, cos, sin, Wq, Wk, Wv, Wproj, Wgate)
    res = run_bass_kernel_spmd(
        nc, maps, core_ids=list(range(NCORES)),
        trace=bool(int(os.environ.get("KERNEL_TRACE", "0"))))
    last_results = res
    out = np.zeros((B, S, E), np.float32)
    for core in range(NCORES):
        out[core // 4] += res.results[core]["outp"]
    return out



# revision 15
# speedup vs baseline: 1.3361x; 1.3361x over previous
"""Trainium2 Bass kernel for nn_CausalSelfAttention_39213051412899.

Sliding-window causal GQA attention with value-embedding gate.
Sharding: 8 cores = batch(2) x kv-group(4).  Each core computes its batch's
4 query heads / 1 kv head and a row-parallel partial of the output
projection; the host sums the 4 partials per batch.

v2: bf16 matmul operands (FWL weight loads), software-pipelined phases so
the PE never idles past the HAM window, post-exp binary masks on GPSIMD,
k-RMS folded into the exp scale AP, single activation-table set
(ln/exp for rsqrt and sigmoid), per-chunk softmax epilogue.

Self-contained: only needs numpy + the concourse tree staged in the
container at /opt/trn_rl_repo (environment, not problem files).
"""

import os
import sys

import numpy as np

try:
    import concourse.bass as bass  # noqa: F401
except ImportError:  # pragma: no cover
    sys.path.insert(0, "/opt/trn_rl_repo")

import ml_dtypes

import concourse.bass as bass
import concourse.tile as tile
from concourse import bacc
from concourse import mybir
from concourse.bass_utils import run_bass_kernel_spmd

F32 = mybir.dt.float32
BF16 = mybir.dt.bfloat16
AF = mybir.ActivationFunctionType
ALU = mybir.AluOpType
NPBF = ml_dtypes.bfloat16

B, S, E = 2, 2048, 1024
H, KV, D = 16, 4, 64
G = H // KV          # 4 q heads per kv head (per core)
GC = 128             # gate channels
EPS = 1.1920929e-07
T = S // 128         # 16 s-tiles
QKVW = G * D + D + D + 2   # 386 packed cols (q|k|v|gate|pad)
GATE_COL = G * D + 2 * D   # 384
NCORES = 8
HT = T // 2          # tiles per half
QT = 4               # tiles per phase-2 group
NQ = T // QT         # 4 groups (2 per half)
VW = D + 2           # vaug row stride (v | ones | pad) - keeps 4B alignment

_cache = {}
last_results = None   # test harness reads exec_time_ns off this


def _block_range(ti, tj, lo_delta, hi_delta):
    """Active query-tile range for key-tile tj (None if empty)."""
    lo = max(ti[0], tj - hi_delta)
    hi = min(ti[1], tj - lo_delta)
    if lo > hi:
        return None
    return lo, hi


def _mask_table(wl, wr_eff):
    """Distinct 0/1 mask tiles needed, keyed (kind, base) -> index."""
    lo_delta = -((127 + wl) // 128)
    hi_delta = (127 + wr_eff) // 128
    keys = {}
    for dt_ in range(lo_delta, hi_delta + 1):   # dt_ = tj - tb
        bw = wl + 128 * dt_
        if -127 <= bw < 127:
            keys.setdefault(("w", bw), len(keys))
        bc = wr_eff - 128 * dt_
        if -127 <= bc < 127:
            keys.setdefault(("c", bc), len(keys))
    return keys


def _mask_tiles(wl, wr_eff):
    keys = _mask_table(wl, wr_eff)
    n = max(1, len(keys))
    m = np.ones((128, n * 128), np.float32)
    rj = np.arange(128)[:, None]
    ri = np.arange(128)[None, :]
    for (kind, base), i in keys.items():
        if kind == "w":
            bad = (rj - ri + base) < 0
        else:
            bad = (ri - rj + base) < 0
        m[:, i * 128:(i + 1) * 128] = np.where(bad, 0.0, 1.0)
    return m.astype(NPBF)


def _build(wl, wr):
    wr_eff = min(int(wr), 0)
    wl = int(wl)
    lo_delta = -((127 + wl) // 128)    # tj - ti >= lo_delta
    hi_delta = (127 + wr_eff) // 128   # tj - ti <= hi_delta  (0 when wr>=0)

    nc = bacc.Bacc(None, target_bir_lowering=False)
    d_xT = nc.declare_dram_parameter("xT", [E, S], BF16, isOutput=False)
    d_wqkv = nc.declare_dram_parameter("wqkv", [E, QKVW], BF16, isOutput=False)
    d_cos4 = nc.declare_dram_parameter("cos4", [128, S], BF16, isOutput=False)
    d_sin4 = nc.declare_dram_parameter("sin4", [128, S], BF16, isOutput=False)
    d_ve2 = nc.declare_dram_parameter("ve2", [128, T * D], BF16, isOutput=False)
    d_wproj = nc.declare_dram_parameter("wproj", [G * D, E], BF16,
                                        isOutput=False)
    d_ident = nc.declare_dram_parameter("ident", [128, 128], BF16,
                                        isOutput=False)
    d_zero = nc.declare_dram_parameter("zero512", [128, 512], BF16,
                                       isOutput=False)
    mask_idx = _mask_table(wl, wr_eff)
    nmask = max(1, len(mask_idx))
    d_masks = nc.declare_dram_parameter("masks", [128, nmask * 128], BF16,
                                        isOutput=False)
    d_out = nc.declare_dram_parameter("outp", [S, E], F32, isOutput=True)

    # chunking: widest chunk for which every chunk has a full-span tj
    def _has_full(cs_tiles):
        for C in range(T // cs_tiles):
            ti_ = (cs_tiles * C, cs_tiles * C + cs_tiles - 1)
            if not any(_block_range(ti_, tj, lo_delta, hi_delta) == ti_
                       for tj in range(T)):
                return False
        return True
    CST = 8 if _has_full(8) else 4       # chunk tiles
    CS = CST * 128
    NCH = T // CST
    pad_mode = not _has_full(CST)

    with tile.TileContext(nc) as tc:
        with tc.tile_pool(name="persist", bufs=1) as persist:
            qT = [persist.tile([128, S], BF16, tag=f"qT{i}", name=f"qT{i}")
                  for i in range(2)]                       # 2 heads per tile
            kT = persist.tile([128, S], BF16)              # k duplicated
            vaug = persist.tile([128, T, VW], BF16)        # v | ones | pad
            yT = [persist.tile([128, S], BF16, tag=f"yT{i}", name=f"yT{i}")
                  for i in range(2)]
            ident = persist.tile([128, 128], BF16)
            masks = persist.tile([128, nmask * 128], BF16)
            cos4 = persist.tile([128, S], BF16)
            sin4 = persist.tile([128, S], BF16)
            ve2 = persist.tile([128, T * D], BF16)
            wq_s = persist.tile([128, 8, QKVW], BF16)
            wp_s = persist.tile([128, 2, E], BF16)
            sigt = persist.tile([128, T], F32)             # sigmoid gate
            gtmp = persist.tile([128, T], F32)
            statq = persist.tile([128, 4 * T], F32)        # q sumsq (t,h)
            statk = persist.tile([128, T], F32)            # k sumsq
            rsq = persist.tile([128, 4 * T], BF16)         # 1/rms(q)
            rskD = persist.tile([128, T], F32)             # D^-.5/rms(k)
            Lt = persist.tile([8, CS], BF16)               # softmax denoms
            Linvb = persist.tile([8, CS], BF16)
            qkvs = [persist.tile([128, HT, QKVW], BF16, tag=f"qkv{i}",
                                 name=f"qkv{i}") for i in range(2)]
            epsq = persist.tile([128, 1], F32)
            epsk = persist.tile([128, 1], F32)
            nc.vector.memset(epsq[:], EPS)
            nc.vector.memset(epsk[:], float(EPS * D))

            # prologue DMAs: phase-2 constants from the scalar queue (idle
            # early), x tiles + weights from sync
            nc.scalar.dma_start(ident[:], d_ident[:, :])
            nc.scalar.dma_start(masks[:], d_masks[:, :])
            nc.scalar.dma_start(cos4[:], d_cos4[:, :])
            nc.scalar.dma_start(sin4[:], d_sin4[:, :])
            nc.scalar.dma_start(ve2[:], d_ve2[:, :])
            for kc in range(2):
                nc.scalar.dma_start(wp_s[:, kc, :],
                                    d_wproj[kc * 128:(kc + 1) * 128, :])
            nc.vector.memset(vaug[:, :, D:D + 1], 1.0)

            # ---------------- phase 1: qkv projections ---------------------
            with (
                tc.tile_pool(name="ph1x", bufs=2) as ph1x,
                tc.tile_pool(name="pq", bufs=3, space="PSUM") as pq,
            ):
                for hf in range(2):
                    xts = ph1x.tile([128, 8, HT * 128], BF16, tag="xts")
                    for c in range(8):
                        if hf == 0:
                            nc.sync.dma_start(wq_s[:, c, :],
                                              d_wqkv[c * 128:(c + 1) * 128, :])
                        nc.sync.dma_start(
                            xts[:, c, :],
                            d_xT[c * 128:(c + 1) * 128,
                                 hf * HT * 128:(hf + 1) * HT * 128])
                    qkv = qkvs[hf]
                    for t in range(HT):
                        ps = pq.tile([128, QKVW], F32)
                        for c in range(8):
                            nc.tensor.matmul(
                                ps[:], xts[:, c, t * 128:(t + 1) * 128],
                                wq_s[:, c, :],
                                start=(c == 0), stop=(c == 7))
                        nc.vector.tensor_copy(qkv[:, t, :], ps[:])

            # ---------------- phase 2 + 3 interleaved ----------------------
            rkq = {}

            def ph2_group(g, ph1t):
                """rope/rms/gate for tiles [4g, 4g+4) (no transposes)."""
                hf = g // 2
                t0 = g * QT                  # global tile base
                l0 = (g % 2) * QT            # local tile base within half
                qkv = qkvs[hf]
                qv = qkv[:, l0:l0 + QT, 0:G * D].rearrange(
                    "p t (h d) -> p t h d", h=G)
                kv_ = qkv[:, l0:l0 + QT, G * D:G * D + D].rearrange(
                    "p t (o d) -> p t o d", o=1)

                # sum-of-squares from pre-rope q/k (rope preserves norms)
                sqq = ph1t.tile([128, QT, G, D], BF16, tag="sqq")
                sqk = ph1t.tile([128, QT, 1, D], BF16, tag="sqk")
                nc.vector.tensor_mul(sqq[:], qv, qv)
                nc.vector.tensor_mul(sqk[:], kv_, kv_)
                nc.vector.tensor_reduce(
                    op=ALU.add,
                    out=statq[:, t0 * 4:(t0 + QT) * 4].rearrange(
                        "p (t h) -> p t h", h=G),
                    in_=sqq[:], axis=mybir.AxisListType.X)
                nc.vector.tensor_reduce(
                    op=ALU.add,
                    out=statk[:, t0:t0 + QT].rearrange(
                        "p (t h) -> p t h", h=1),
                    in_=sqk[:], axis=mybir.AxisListType.X)
                # 1/sqrt via exp(-0.5*ln(.)) - stays in the ln/exp table set
                qs = slice(t0 * 4, (t0 + QT) * 4)
                nc.scalar.activation(statq[:, qs], statq[:, qs], AF.Ln,
                                     bias=epsq[:], scale=1.0 / D)
                nc.scalar.activation(rsq[:, qs], statq[:, qs], AF.Exp,
                                     scale=-0.5)
                ks = slice(t0, t0 + QT)
                nc.scalar.activation(statk[:, ks], statk[:, ks], AF.Ln,
                                     bias=epsk[:], scale=1.0)
                nc.scalar.activation(rskD[:, ks], statk[:, ks], AF.Exp,
                                     scale=-0.5)

                # rope
                cosq = cos4[:, t0 * 128:(t0 + QT) * 128].rearrange(
                    "p (t h f) -> p t h f", h=G, f=32)
                sinq = sin4[:, t0 * 128:(t0 + QT) * 128].rearrange(
                    "p (t h f) -> p t h f", h=G, f=32)
                rq = ph1t.tile([128, QT, G, D], BF16, tag="rq")
                rk = ph1t.tile([128, QT, 1, D], BF16, tag="rk")
                rkq[g] = rk
                tmp = ph1t.tile([128, QT, G, 32], BF16, tag="tmp")
                tmpk = ph1t.tile([128, QT, 1, 32], BF16, tag="tmpk")
                for (src, dst, cs, sn, tm) in (
                        (qv, rq, cosq, sinq, tmp),
                        (kv_, rk, cosq[:, :, 0:1, :], sinq[:, :, 0:1, :],
                         tmpk)):
                    x1 = src[:, :, :, 0:32]
                    x2 = src[:, :, :, 32:64]
                    nc.vector.tensor_mul(dst[:, :, :, 0:32], x1, cs)
                    nc.vector.tensor_mul(tm[:], x2, sn)
                    nc.vector.tensor_add(
                        dst[:, :, :, 0:32], dst[:, :, :, 0:32], tm[:])
                    nc.vector.tensor_mul(dst[:, :, :, 32:64], x2, cs)
                    nc.vector.tensor_mul(tm[:], x1, sn)
                    nc.vector.tensor_sub(
                        dst[:, :, :, 32:64], dst[:, :, :, 32:64], tm[:])

                # normalize q back into qkv (k stays raw; rms in exp scale)
                rsb = rsq[:, qs].rearrange("p (t h o) -> p t h o", h=G, o=1)
                nc.vector.tensor_mul(qv, rq[:], rsb.to_broadcast(
                    [128, QT, G, D]))

                # v + sig * ve2  (ve2 pre-scaled by 2 on host)
                tmpv = ph1t.tile([128, QT, D], BF16, tag="tmpv")
                for t in range(QT):
                    tt = t0 + t
                    nc.vector.tensor_scalar_mul(
                        tmpv[:, t, :], ve2[:, tt * D:(tt + 1) * D],
                        sigt[:, tt:tt + 1])
                nc.vector.tensor_add(
                    vaug[:, t0:t0 + QT, 0:D],
                    qkv[:, l0:l0 + QT, G * D + D:G * D + 2 * D], tmpv[:])

            def ph2_transpose(g, ptr):
                hf = g // 2
                t0 = g * QT
                l0 = (g % 2) * QT
                qkv = qkvs[hf]
                for t in range(QT):
                    tt = t0 + t
                    for bk in range(2):
                        tp = ptr.tile([128, 128], BF16, tag="tp", name="tp")
                        nc.tensor.transpose(
                            tp[:], qkv[:, l0 + t, bk * 128:(bk + 1) * 128],
                            ident[:])
                        nc.vector.tensor_copy(
                            qT[bk][:, tt * 128:(tt + 1) * 128], tp[:])
                    tp = ptr.tile([128, 128], BF16, tag="tp", name="tp")
                    nc.tensor.transpose(
                        tp[0:64, :],
                        rkq[g][:, t, 0, :],
                        ident[:])
                    nc.vector.tensor_copy(
                        kT[0:64, tt * 128:(tt + 1) * 128], tp[0:64, :])

            def gate_exp(hf):
                qkv = qkvs[hf]
                t0 = hf * HT
                # sigmoid(z) = 1/(1+exp(-z)) - keeps the ln/exp table set
                nc.scalar.activation(
                    gtmp[:, t0:t0 + HT],
                    qkv[:, :, GATE_COL:GATE_COL + 1].rearrange(
                        "p t o -> p (t o)"),
                    AF.Exp, scale=-1.0)
                nc.vector.tensor_scalar_add(
                    gtmp[:, t0:t0 + HT], gtmp[:, t0:t0 + HT], 1.0)
                nc.vector.reciprocal(sigt[:, t0:t0 + HT], gtmp[:, t0:t0 + HT])

            ytus = {}

            def stream(h, C, att, psc, ppv, pytu):
                """One (head, chunk) attention stream."""
                rh = slice((h % 2) * 64, (h % 2) * 64 + 64)
                qTh = qT[h // 2]
                c0 = CST * C
                ti = (c0, c0 + CST - 1)
                tjs = [tj for tj in
                       range(max(0, c0 + lo_delta),
                             min(T - 1, c0 + CST - 1 + hi_delta) + 1)
                       if _block_range(ti, tj, lo_delta, hi_delta)]
                full = [tj for tj in tjs
                        if _block_range(ti, tj, lo_delta, hi_delta) == ti]
                if pad_mode:
                    order = tjs
                else:
                    ftj = full[-1]
                    order = [ftj] + [tj for tj in tjs if tj != ftj]

                yTa = ppv.tile([65, CS], F32, tag="yTa", name="yTa")
                half_started = [False] * (CST // 4)
                half_last = {}
                for i, tj in enumerate(order):
                    lo_, hi_ = _block_range(ti, tj, lo_delta, hi_delta)
                    o_, n_ = ((0, CS) if (pad_mode and i == 0) else
                              ((lo_ - c0) * 128, (hi_ - lo_ + 1) * 128))
                    for hx in range(CST // 4):
                        if o_ < (hx + 1) * 512 and o_ + n_ > hx * 512:
                            half_last[hx] = i
                pend = []

                def emit_pv(rec):
                    i, tj, pt, off, n = rec
                    for hx in range(CST // 4):
                        h0_ = hx * 512
                        s0 = max(off, h0_)
                        s1 = min(off + n, h0_ + 512)
                        if s0 >= s1:
                            continue
                        first = not half_started[hx]
                        half_started[hx] = True
                        nc.tensor.matmul(
                            yTa[:, s0:s1], vaug[:, tj, 0:D + 1],
                            pt[:, s0:s1],
                            start=first, stop=(half_last[hx] == i))

                for i, tj in enumerate(order):
                    alo, ahi = _block_range(ti, tj, lo_delta, hi_delta)
                    aoff = (alo - c0) * 128
                    an = (ahi - alo + 1) * 128
                    if pad_mode and i == 0:
                        off, n = 0, CS
                    else:
                        off, n = aoff, an
                    sc = psc.tile([128, CS], F32, tag="sc", name="sc")
                    pt = att.tile([128, CS], BF16, tag="pt", name="pt")
                    p0 = aoff
                    while p0 < aoff + an:
                        p1 = min((p0 // 512 + 1) * 512, aoff + an)
                        nc.tensor.matmul(
                            sc[:, p0:p1],
                            kT[rh, tj * 128:(tj + 1) * 128],
                            qTh[rh, C * CS + p0:C * CS + p1],
                            start=True, stop=True)
                        p0 = p1
                    if pad_mode:
                        nc.sync.dma_start(
                            pt[:], d_zero[:, :].to_broadcast([128, CS]))
                    nc.scalar.activation(
                        pt[:, aoff:aoff + an], sc[:, aoff:aoff + an],
                        AF.Exp, scale=rskD[:, tj:tj + 1])
                    # post-exp 0/1 masks on gpsimd
                    for tb in range(alo, ahi + 1):
                        bo = (tb - c0) * 128
                        for kind, base in (("w", wl - 128 * (tb - tj)),
                                           ("c", wr_eff + 128 * (tb - tj))):
                            if -127 <= base < 127:
                                mi = mask_idx[(kind, base)]
                                nc.gpsimd.tensor_mul(
                                    pt[:, bo:bo + 128],
                                    pt[:, bo:bo + 128],
                                    masks[:, mi * 128:(mi + 1) * 128])
                    pend.append((i, tj, pt, off, n))
                    if len(pend) > 1:
                        emit_pv(pend.pop(0))
                emit_pv(pend.pop(0))

                ytu = pytu.tile([65, CS], BF16, tag="ytu", name="ytu")
                nc.vector.tensor_copy(ytu[:], yTa[:])
                rr = C * G + h
                nc.sync.dma_start(Lt[rr:rr + 1, 0:CS], ytu[64:65, :])
                ytus[(h, C)] = ytu

            with (
                tc.tile_pool(name="ph1t", bufs=2) as ph1t,
                tc.tile_pool(name="att", bufs=6) as att,
                tc.tile_pool(name="ytu", bufs=G * NCH) as pytu,
                tc.tile_pool(name="lbp", bufs=4) as plb,
                tc.tile_pool(name="ytn", bufs=2) as pytn,
                tc.tile_pool(name="ptr", bufs=2, space="PSUM") as ptr,
                tc.tile_pool(name="psc", bufs=2, space="PSUM") as psc,
                tc.tile_pool(name="ppv", bufs=1, space="PSUM") as ppv,
                tc.tile_pool(name="dsc", bufs=1, space="DRAM") as dsc,
            ):
                d_linv = dsc.tile([G * NCH, CS], BF16)

                # --- half 0: gate, groups 0-1, transposes, kT dup ---
                gate_exp(0)
                for g in (0, 1):
                    ph2_group(g, ph1t)
                    ph2_transpose(g, ptr)
                nc.sync.dma_start(kT[64:128, 0:HT * 128],
                                  kT[0:64, 0:HT * 128])

                # --- half 1 DVE/ACT/GPSIMD work (no transposes yet) ---
                gate_exp(1)
                for g in (2, 3):
                    ph2_group(g, ph1t)

                # --- chunk 0 attention (needs only half-0 tiles) ---
                for h in range(G):
                    stream(h, 0, att, psc, ppv, pytu)

                # --- half 1 transposes + kT dup ---
                for g in (2, 3):
                    ph2_transpose(g, ptr)
                nc.sync.dma_start(kT[64:128, HT * 128:S],
                                  kT[0:64, HT * 128:S])

                # --- remaining chunks ---
                for C in range(1, NCH):
                    for h in range(G):
                        stream(h, C, att, psc, ppv, pytu)

                # --- softmax denominators -> yT ---
                with nc.allow_low_precision(
                        reason="1/L in bf16; 0.4% on softmax denom is fine "
                               "for the 2e-2 budget"):
                    nc.vector.reciprocal(Linvb[0:G * NCH, 0:CS],
                                         Lt[0:G * NCH, 0:CS])
                nc.sync.dma_start(d_linv[:], Linvb[0:G * NCH, 0:CS])
                for C in range(NCH):
                    for h in range(G):
                        rr = C * G + h
                        ytu = ytus[(h, C)]
                        lb = plb.tile([64, CS], BF16)
                        nc.sync.dma_start(
                            lb[:],
                            d_linv[rr:rr + 1, :].to_broadcast([64, CS]))
                        ccols = slice(C * CS, (C + 1) * CS)
                        if h % 2 == 0:
                            nc.vector.tensor_mul(
                                yT[h // 2][0:64, ccols], ytu[0:64, :], lb[:])
                        else:
                            ytn = pytn.tile([64, CS], BF16)
                            nc.vector.tensor_mul(ytn[:], ytu[0:64, :], lb[:])
                            nc.sync.dma_start(yT[h // 2][64:128, ccols],
                                              ytn[:])

            # ---------------- phase 4: output projection -------------------
            with (
                tc.tile_pool(name="ob", bufs=3) as pob,
                tc.tile_pool(name="po", bufs=4, space="PSUM") as ppo,
            ):
                for t in range(T):
                    ob = pob.tile([128, E], F32)
                    for nh in range(2):
                        po = ppo.tile([128, 512], F32)
                        nc.tensor.matmul(
                            po[:], yT[0][:, t * 128:(t + 1) * 128],
                            wp_s[:, 0, nh * 512:(nh + 1) * 512],
                            start=True, stop=False)
                        nc.tensor.matmul(
                            po[:], yT[1][:, t * 128:(t + 1) * 128],
                            wp_s[:, 1, nh * 512:(nh + 1) * 512],
                            start=False, stop=True)
                        if (t + nh) % 2 == 0:
                            nc.vector.tensor_copy(
                                ob[:, nh * 512:(nh + 1) * 512], po[:])
                        else:
                            nc.scalar.copy(
                                ob[:, nh * 512:(nh + 1) * 512], po[:])
                    nc.sync.dma_start(
                        d_out[t * 128:(t + 1) * 128, :], ob[:])
    nc.compile()
    return nc


def _prep_inputs(x, ve, cos, sin, Wq, Wk, Wv, Wproj, Wgate):
    cosn = np.asarray(cos, np.float32).reshape(S, 32)
    sinn = np.asarray(sin, np.float32).reshape(S, 32)
    cos4 = np.empty((128, S), np.float32)
    sin4 = np.empty((128, S), np.float32)
    for t in range(T):
        cos4[:, t * 128:(t + 1) * 128] = np.tile(
            cosn[t * 128:(t + 1) * 128], (1, 4))
        sin4[:, t * 128:(t + 1) * 128] = np.tile(
            sinn[t * 128:(t + 1) * 128], (1, 4))
    cos4 = cos4.astype(NPBF)
    sin4 = sin4.astype(NPBF)

    Wq = np.asarray(Wq, np.float32)
    Wk = np.asarray(Wk, np.float32)
    Wv = np.asarray(Wv, np.float32)
    Wproj = np.asarray(Wproj, np.float32)
    Wgate = np.asarray(Wgate, np.float32)
    maps = []
    wl_ = int(getattr(_prep_inputs, '_wl', 1024))
    wr_ = min(int(getattr(_prep_inputs, '_wr', 0)), 0)
    maskt = _mask_tiles(wl_, wr_)
    for core in range(NCORES):
        b, g = core // 4, core % 4
        xT = np.ascontiguousarray(
            np.asarray(x[b], np.float32).T).astype(NPBF)
        wg = np.zeros((E, 1), np.float32)
        wg[:GC, 0] = Wgate[:, g]
        wqkv = np.ascontiguousarray(np.concatenate([
            Wq[:, g * G * D:(g + 1) * G * D],
            Wk[:, g * D:(g + 1) * D],
            Wv[:, g * D:(g + 1) * D],
            wg, np.zeros((E, 1), np.float32)], axis=1)).astype(NPBF)
        veg = 2.0 * np.asarray(ve[b][:, g * D:(g + 1) * D], np.float32)
        ve2 = np.ascontiguousarray(
            veg.reshape(T, 128, D).transpose(1, 0, 2).reshape(
                128, T * D)).astype(NPBF)
        wproj = np.ascontiguousarray(
            Wproj[g * G * D:(g + 1) * G * D, :]).astype(NPBF)
        maps.append({"xT": xT, "wqkv": wqkv, "cos4": cos4, "sin4": sin4,
                     "ve2": ve2, "wproj": wproj,
                     "ident": np.eye(128).astype(NPBF),
                     "masks": maskt,
                     "zero512": np.zeros((128, 512), NPBF)})
    return maps


def kernel(x, ve, cos, sin, Wq, Wk, Wv, Wproj, Wgate,
           window_left, window_right):
    global last_results
    wl, wr = int(window_left), int(window_right)
    key = (wl, wr)
    if key not in _cache:
        _cache[key] = _build(wl, wr)
    nc = _cache[key]
    _prep_inputs._wl, _prep_inputs._wr = wl, wr
    maps = _prep_inputs(x, ve, cos, sin, Wq, Wk, Wv, Wproj, Wgate)
    res = run_bass_kernel_spmd(
        nc, maps, core_ids=list(range(NCORES)),
        trace=bool(int(os.environ.get("KERNEL_TRACE", "0"))))
    last_results = res
    out = np.zeros((B, S, E), np.float32)
    for core in range(NCORES):
        out[core // 4] += res.results[core]["outp"]
    return out


# revision 18
# speedup vs baseline: 1.3365x; 1.0004x over previous
"""Trainium2 Bass kernel for nn_CausalSelfAttention_39213051412899.

Sliding-window causal GQA attention with value-embedding gate.
Sharding: 8 cores = batch(2) x kv-group(4).  Each core computes its batch's
4 query heads / 1 kv head and a row-parallel partial of the output
projection; the host sums the 4 partials per batch.

v2: bf16 matmul operands (FWL weight loads), software-pipelined phases so
the PE never idles past the HAM window, post-exp binary masks on GPSIMD,
k-RMS folded into the exp scale AP, single activation-table set
(ln/exp for rsqrt and sigmoid), per-chunk softmax epilogue.

Self-contained: only needs numpy + the concourse tree staged in the
container at /opt/trn_rl_repo (environment, not problem files).
"""

import os
import sys

import numpy as np

try:
    import concourse.bass as bass  # noqa: F401
except ImportError:  # pragma: no cover
    sys.path.insert(0, "/opt/trn_rl_repo")

import ml_dtypes

import concourse.bass as bass
import concourse.tile as tile
from concourse import bacc
from concourse import mybir
from concourse.bass_utils import run_bass_kernel_spmd

F32 = mybir.dt.float32
BF16 = mybir.dt.bfloat16
AF = mybir.ActivationFunctionType
ALU = mybir.AluOpType
NPBF = ml_dtypes.bfloat16

B, S, E = 2, 2048, 1024
H, KV, D = 16, 4, 64
G = H // KV          # 4 q heads per kv head (per core)
GC = 128             # gate channels
EPS = 1.1920929e-07
T = S // 128         # 16 s-tiles
QKVW = G * D + D + D + 2   # 386 packed cols (q|k|v|gate|pad)
GATE_COL = G * D + 2 * D   # 384
NCORES = 8
HT = T // 2          # tiles per half
QT = 4               # tiles per phase-2 group
NQ = T // QT         # 4 groups (2 per half)
VW = D + 2           # vaug row stride (v | ones | pad) - keeps 4B alignment

_cache = {}
last_results = None   # test harness reads exec_time_ns off this


def _block_range(ti, tj, lo_delta, hi_delta):
    """Active query-tile range for key-tile tj (None if empty)."""
    lo = max(ti[0], tj - hi_delta)
    hi = min(ti[1], tj - lo_delta)
    if lo > hi:
        return None
    return lo, hi


def _mask_table(wl, wr_eff):
    """Distinct 0/1 mask tiles needed, keyed (kind, base) -> index."""
    lo_delta = -((127 + wl) // 128)
    hi_delta = (127 + wr_eff) // 128
    keys = {}
    for dt_ in range(lo_delta, hi_delta + 1):   # dt_ = tj - tb
        bw = wl + 128 * dt_
        if -127 <= bw < 127:
            keys.setdefault(("w", bw), len(keys))
        bc = wr_eff - 128 * dt_
        if -127 <= bc < 127:
            keys.setdefault(("c", bc), len(keys))
    return keys


def _mask_tiles(wl, wr_eff):
    keys = _mask_table(wl, wr_eff)
    n = max(1, len(keys))
    m = np.ones((128, n * 128), np.float32)
    rj = np.arange(128)[:, None]
    ri = np.arange(128)[None, :]
    for (kind, base), i in keys.items():
        if kind == "w":
            bad = (rj - ri + base) < 0
        else:
            bad = (ri - rj + base) < 0
        m[:, i * 128:(i + 1) * 128] = np.where(bad, 0.0, 1.0)
    return m.astype(NPBF)


def _build(wl, wr):
    wr_eff = min(int(wr), 0)
    wl = int(wl)
    lo_delta = -((127 + wl) // 128)    # tj - ti >= lo_delta
    hi_delta = (127 + wr_eff) // 128   # tj - ti <= hi_delta  (0 when wr>=0)

    nc = bacc.Bacc(None, target_bir_lowering=False)
    d_xT = nc.declare_dram_parameter("xT", [E, S], BF16, isOutput=False)
    d_wqkv = nc.declare_dram_parameter("wqkv", [E, QKVW], BF16, isOutput=False)
    d_cos4 = nc.declare_dram_parameter("cos4", [128, S], BF16, isOutput=False)
    d_sin4 = nc.declare_dram_parameter("sin4", [128, S], BF16, isOutput=False)
    d_ve2 = nc.declare_dram_parameter("ve2", [128, T * D], BF16, isOutput=False)
    d_wproj = nc.declare_dram_parameter("wproj", [G * D, E], BF16,
                                        isOutput=False)
    d_ident = nc.declare_dram_parameter("ident", [128, 128], BF16,
                                        isOutput=False)
    d_zero = nc.declare_dram_parameter("zero512", [128, 512], BF16,
                                       isOutput=False)
    mask_idx = _mask_table(wl, wr_eff)
    nmask = max(1, len(mask_idx))
    d_masks = nc.declare_dram_parameter("masks", [128, nmask * 128], BF16,
                                        isOutput=False)
    d_out = nc.declare_dram_parameter("outp", [S, E], F32, isOutput=True)

    # chunking: widest chunk for which every chunk has a full-span tj
    def _has_full(cs_tiles):
        for C in range(T // cs_tiles):
            ti_ = (cs_tiles * C, cs_tiles * C + cs_tiles - 1)
            if not any(_block_range(ti_, tj, lo_delta, hi_delta) == ti_
                       for tj in range(T)):
                return False
        return True
    CST = 8 if _has_full(8) else 4       # chunk tiles
    CS = CST * 128
    NCH = T // CST
    pad_mode = not _has_full(CST)

    with tile.TileContext(nc) as tc:
        with tc.tile_pool(name="persist", bufs=1) as persist:
            qT = [persist.tile([128, S], BF16, tag=f"qT{i}", name=f"qT{i}")
                  for i in range(2)]                       # 2 heads per tile
            kT = persist.tile([128, S], BF16)              # k duplicated
            vaug = persist.tile([128, T, VW], BF16)        # v | ones | pad
            yT = [persist.tile([128, S], BF16, tag=f"yT{i}", name=f"yT{i}")
                  for i in range(2)]
            ident = persist.tile([128, 128], BF16)
            masks = persist.tile([128, nmask * 128], BF16)
            cos4 = persist.tile([128, S], BF16)
            sin4 = persist.tile([128, S], BF16)
            ve2 = persist.tile([128, T * D], BF16)
            wq_s = persist.tile([128, 8, QKVW], BF16)
            wp_s = persist.tile([128, 2, E], BF16)
            sigt = persist.tile([128, T], F32)             # sigmoid gate
            gtmp = persist.tile([128, T], F32)
            statq = persist.tile([128, 4 * T], F32)        # q sumsq (t,h)
            statk = persist.tile([128, T], F32)            # k sumsq
            rsq = persist.tile([128, 4 * T], BF16)         # 1/rms(q)
            rskD = persist.tile([128, T], F32)             # D^-.5/rms(k)
            Lt = persist.tile([64, CS], BF16)              # softmax denoms
            Linvb = persist.tile([64, CS], BF16)           # chunk C rows at 32C
            qkvs = [persist.tile([128, HT, QKVW], BF16, tag=f"qkv{i}",
                                 name=f"qkv{i}") for i in range(2)]
            epsq = persist.tile([128, 1], F32)
            epsk = persist.tile([128, 1], F32)
            nc.vector.memset(epsq[:], EPS)
            nc.vector.memset(epsk[:], float(EPS * D))

            # prologue DMAs: phase-2 constants from the scalar queue (idle
            # early), x tiles + weights from sync
            nc.scalar.dma_start(ident[:], d_ident[:, :])
            nc.scalar.dma_start(masks[:], d_masks[:, :])
            nc.scalar.dma_start(cos4[:], d_cos4[:, :])
            nc.scalar.dma_start(sin4[:], d_sin4[:, :])
            nc.scalar.dma_start(ve2[:], d_ve2[:, :])
            for kc in range(2):
                nc.scalar.dma_start(wp_s[:, kc, :],
                                    d_wproj[kc * 128:(kc + 1) * 128, :])
            nc.vector.memset(vaug[:, :, D:D + 1], 1.0)

            # ---------------- phase 1: qkv projections ---------------------
            with (
                tc.tile_pool(name="ph1x", bufs=2) as ph1x,
                tc.tile_pool(name="pq", bufs=3, space="PSUM") as pq,
            ):
                for hf in range(2):
                    xts = ph1x.tile([128, 8, HT * 128], BF16, tag="xts")
                    for c in range(8):
                        if hf == 0:
                            nc.sync.dma_start(wq_s[:, c, :],
                                              d_wqkv[c * 128:(c + 1) * 128, :])
                        if hf == 0:
                            hw = HT * 128 // 2
                            for sb in range(2):
                                nc.sync.dma_start(
                                    xts[:, c, sb * hw:(sb + 1) * hw],
                                    d_xT[c * 128:(c + 1) * 128,
                                         sb * hw:(sb + 1) * hw])
                        else:
                            nc.sync.dma_start(
                                xts[:, c, :],
                                d_xT[c * 128:(c + 1) * 128,
                                     hf * HT * 128:(hf + 1) * HT * 128])
                    qkv = qkvs[hf]
                    for t in range(HT):
                        ps = pq.tile([128, QKVW], F32)
                        for c in range(8):
                            nc.tensor.matmul(
                                ps[:], xts[:, c, t * 128:(t + 1) * 128],
                                wq_s[:, c, :],
                                start=(c == 0), stop=(c == 7))
                        nc.vector.tensor_copy(qkv[:, t, :], ps[:])

            # ---------------- phase 2 + 3 interleaved ----------------------
            rkq = {}

            def ph2_group(g, ph1t):
                """rope/rms/gate for tiles [4g, 4g+4) (no transposes)."""
                hf = g // 2
                t0 = g * QT                  # global tile base
                l0 = (g % 2) * QT            # local tile base within half
                qkv = qkvs[hf]
                qv = qkv[:, l0:l0 + QT, 0:G * D].rearrange(
                    "p t (h d) -> p t h d", h=G)
                kv_ = qkv[:, l0:l0 + QT, G * D:G * D + D].rearrange(
                    "p t (o d) -> p t o d", o=1)

                # sum-of-squares from pre-rope q/k (rope preserves norms)
                sqq = ph1t.tile([128, QT, G, D], BF16, tag="sqq")
                sqk = ph1t.tile([128, QT, 1, D], BF16, tag="sqk")
                nc.vector.tensor_mul(sqq[:], qv, qv)
                nc.vector.tensor_mul(sqk[:], kv_, kv_)
                nc.vector.tensor_reduce(
                    op=ALU.add,
                    out=statq[:, t0 * 4:(t0 + QT) * 4].rearrange(
                        "p (t h) -> p t h", h=G),
                    in_=sqq[:], axis=mybir.AxisListType.X)
                nc.vector.tensor_reduce(
                    op=ALU.add,
                    out=statk[:, t0:t0 + QT].rearrange(
                        "p (t h) -> p t h", h=1),
                    in_=sqk[:], axis=mybir.AxisListType.X)
                # 1/sqrt via Sqrt + DVE reciprocal
                qs = slice(t0 * 4, (t0 + QT) * 4)
                nc.scalar.activation(statq[:, qs], statq[:, qs], AF.Sqrt,
                                     bias=epsq[:], scale=1.0 / D)
                with nc.allow_low_precision(
                        reason="bf16 rms scale, 0.4% ok for 2e-2 budget"):
                    nc.vector.reciprocal(rsq[:, qs], statq[:, qs])
                ks = slice(t0, t0 + QT)
                nc.scalar.activation(statk[:, ks], statk[:, ks], AF.Sqrt,
                                     bias=epsk[:], scale=1.0)
                nc.vector.reciprocal(rskD[:, ks], statk[:, ks])

                # rope
                cosq = cos4[:, t0 * 128:(t0 + QT) * 128].rearrange(
                    "p (t h f) -> p t h f", h=G, f=32)
                sinq = sin4[:, t0 * 128:(t0 + QT) * 128].rearrange(
                    "p (t h f) -> p t h f", h=G, f=32)
                rq = ph1t.tile([128, QT, G, D], BF16, tag="rq")
                rk = ph1t.tile([128, QT, 1, D], BF16, tag="rk")
                rkq[g] = rk
                tmp = ph1t.tile([128, QT, G, 32], BF16, tag="tmp")
                tmpk = ph1t.tile([128, QT, 1, 32], BF16, tag="tmpk")
                for (src, dst, cs, sn, tm) in (
                        (qv, rq, cosq, sinq, tmp),
                        (kv_, rk, cosq[:, :, 0:1, :], sinq[:, :, 0:1, :],
                         tmpk)):
                    x1 = src[:, :, :, 0:32]
                    x2 = src[:, :, :, 32:64]
                    nc.vector.tensor_mul(dst[:, :, :, 0:32], x1, cs)
                    nc.vector.tensor_mul(tm[:], x2, sn)
                    nc.vector.tensor_add(
                        dst[:, :, :, 0:32], dst[:, :, :, 0:32], tm[:])
                    nc.vector.tensor_mul(dst[:, :, :, 32:64], x2, cs)
                    nc.vector.tensor_mul(tm[:], x1, sn)
                    nc.vector.tensor_sub(
                        dst[:, :, :, 32:64], dst[:, :, :, 32:64], tm[:])

                # normalize q back into qkv (k stays raw; rms in exp scale)
                rsb = rsq[:, qs].rearrange("p (t h o) -> p t h o", h=G, o=1)
                nc.vector.tensor_mul(qv, rq[:], rsb.to_broadcast(
                    [128, QT, G, D]))

                # v + sig * ve2  (ve2 pre-scaled by 2 on host)
                tmpv = ph1t.tile([128, QT, D], BF16, tag="tmpv")
                for t in range(QT):
                    tt = t0 + t
                    nc.vector.tensor_scalar_mul(
                        tmpv[:, t, :], ve2[:, tt * D:(tt + 1) * D],
                        sigt[:, tt:tt + 1])
                nc.vector.tensor_add(
                    vaug[:, t0:t0 + QT, 0:D],
                    qkv[:, l0:l0 + QT, G * D + D:G * D + 2 * D], tmpv[:])

            def ph2_transpose(g, ptr):
                hf = g // 2
                t0 = g * QT
                l0 = (g % 2) * QT
                qkv = qkvs[hf]
                for t in range(QT):
                    tt = t0 + t
                    for bk in range(2):
                        tp = ptr.tile([128, 128], BF16, tag="tp", name="tp")
                        nc.tensor.transpose(
                            tp[:], qkv[:, l0 + t, bk * 128:(bk + 1) * 128],
                            ident[:])
                        nc.vector.tensor_copy(
                            qT[bk][:, tt * 128:(tt + 1) * 128], tp[:])
                    tp = ptr.tile([128, 128], BF16, tag="tp", name="tp")
                    nc.tensor.transpose(
                        tp[0:64, :],
                        rkq[g][:, t, 0, :],
                        ident[:])
                    nc.vector.tensor_copy(
                        kT[0:64, tt * 128:(tt + 1) * 128], tp[0:64, :])

            def gate_sig(hf):
                qkv = qkvs[hf]
                t0 = hf * HT
                nc.scalar.activation(
                    sigt[:, t0:t0 + HT],
                    qkv[:, :, GATE_COL:GATE_COL + 1].rearrange(
                        "p t o -> p (t o)"),
                    AF.Sigmoid)

            ytus = {}

            def stream(h, C, att, psc, ppv, pytu):
                """One (head, chunk) attention stream."""
                rh = slice((h % 2) * 64, (h % 2) * 64 + 64)
                qTh = qT[h // 2]
                c0 = CST * C
                ti = (c0, c0 + CST - 1)
                tjs = [tj for tj in
                       range(max(0, c0 + lo_delta),
                             min(T - 1, c0 + CST - 1 + hi_delta) + 1)
                       if _block_range(ti, tj, lo_delta, hi_delta)]
                full = [tj for tj in tjs
                        if _block_range(ti, tj, lo_delta, hi_delta) == ti]
                if pad_mode:
                    order = tjs
                else:
                    ftj = full[-1]
                    order = [ftj] + [tj for tj in tjs if tj != ftj]

                yTa = ppv.tile([65, CS], F32, tag="yTa", name="yTa")
                half_started = [False] * (CST // 4)
                half_last = {}
                for i, tj in enumerate(order):
                    lo_, hi_ = _block_range(ti, tj, lo_delta, hi_delta)
                    o_, n_ = ((0, CS) if (pad_mode and i == 0) else
                              ((lo_ - c0) * 128, (hi_ - lo_ + 1) * 128))
                    for hx in range(CST // 4):
                        if o_ < (hx + 1) * 512 and o_ + n_ > hx * 512:
                            half_last[hx] = i
                pend = []

                def emit_pv(rec):
                    i, tj, pt, off, n = rec
                    for hx in range(CST // 4):
                        h0_ = hx * 512
                        s0 = max(off, h0_)
                        s1 = min(off + n, h0_ + 512)
                        if s0 >= s1:
                            continue
                        first = not half_started[hx]
                        half_started[hx] = True
                        nc.tensor.matmul(
                            yTa[:, s0:s1], vaug[:, tj, 0:D + 1],
                            pt[:, s0:s1],
                            start=first, stop=(half_last[hx] == i))

                for i, tj in enumerate(order):
                    alo, ahi = _block_range(ti, tj, lo_delta, hi_delta)
                    aoff = (alo - c0) * 128
                    an = (ahi - alo + 1) * 128
                    if pad_mode and i == 0:
                        off, n = 0, CS
                    else:
                        off, n = aoff, an
                    sc = psc.tile([128, CS], F32, tag="sc", name="sc")
                    p0 = aoff
                    while p0 < aoff + an:
                        p1 = min((p0 // 512 + 1) * 512, aoff + an)
                        nc.tensor.matmul(
                            sc[:, p0:p1],
                            kT[rh, tj * 128:(tj + 1) * 128],
                            qTh[rh, C * CS + p0:C * CS + p1],
                            start=True, stop=True)
                        p0 = p1
                    yield
                    pt = att.tile([128, CS], BF16, tag="pt", name="pt")
                    if pad_mode:
                        nc.sync.dma_start(
                            pt[:], d_zero[:, :].to_broadcast([128, CS]))
                    nc.scalar.activation(
                        pt[:, aoff:aoff + an], sc[:, aoff:aoff + an],
                        AF.Exp, scale=rskD[:, tj:tj + 1])
                    # post-exp 0/1 masks on gpsimd
                    for tb in range(alo, ahi + 1):
                        bo = (tb - c0) * 128
                        for kind, base in (("w", wl - 128 * (tb - tj)),
                                           ("c", wr_eff + 128 * (tb - tj))):
                            if -127 <= base < 127:
                                mi = mask_idx[(kind, base)]
                                nc.vector.tensor_mul(
                                    pt[:, bo:bo + 128],
                                    pt[:, bo:bo + 128],
                                    masks[:, mi * 128:(mi + 1) * 128])
                    pend.append((i, tj, pt, off, n))
                    if len(pend) > 1:
                        emit_pv(pend.pop(0))
                    yield
                emit_pv(pend.pop(0))

                ytu = pytu.tile([65, CS], BF16, tag="ytu", name="ytu")
                nc.vector.tensor_copy(ytu[:], yTa[:])
                rr = C * 32 + h
                nc.sync.dma_start(Lt[rr:rr + 1, 0:CS], ytu[64:65, :])
                ytus[(h, C)] = ytu

            with (
                tc.tile_pool(name="ph1t", bufs=2) as ph1t,
                tc.tile_pool(name="att", bufs=6) as att,
                tc.tile_pool(name="ytu", bufs=G * NCH) as pytu,
                tc.tile_pool(name="lbp", bufs=4) as plb,
                tc.tile_pool(name="ytn", bufs=2) as pytn,
                tc.tile_pool(name="dsc", bufs=1, space="DRAM") as dsc,
            ):
                d_linv = dsc.tile([G * NCH, CS], BF16)

                def epilogue(C):
                    a0 = C * 32
                    with nc.allow_low_precision(
                            reason="1/L in bf16; 0.4% on softmax denom is "
                                   "fine for the 2e-2 budget"):
                        nc.vector.reciprocal(Linvb[a0:a0 + G, 0:CS],
                                             Lt[a0:a0 + G, 0:CS])
                    nc.sync.dma_start(d_linv[C * G:(C + 1) * G, :],
                                      Linvb[a0:a0 + G, 0:CS])
                    ccols = slice(C * CS, (C + 1) * CS)
                    for h in range(G):
                        rr = C * G + h
                        ytu = ytus[(h, C)]
                        lb = plb.tile([64, CS], BF16)
                        nc.sync.dma_start(
                            lb[:],
                            d_linv[rr:rr + 1, :].to_broadcast([64, CS]))
                        if h % 2 == 0:
                            nc.vector.tensor_mul(
                                yT[h // 2][0:64, ccols], ytu[0:64, :], lb[:])
                        else:
                            ytn = pytn.tile([64, CS], BF16)
                            nc.vector.tensor_mul(ytn[:], ytu[0:64, :], lb[:])
                            nc.sync.dma_start(yT[h // 2][64:128, ccols],
                                              ytn[:])

                # --- scope 1: phase 2 + chunk 0 (single streams) ---
                gate_sig(0)
                gate_sig(1)
                with (
                    tc.tile_pool(name="ptr", bufs=2, space="PSUM") as ptr,
                    tc.tile_pool(name="psc", bufs=2, space="PSUM") as psc,
                    tc.tile_pool(name="ppv", bufs=1, space="PSUM") as ppv,
                ):
                    for g in (0, 1):
                        ph2_group(g, ph1t)
                        ph2_transpose(g, ptr)
                    nc.sync.dma_start(kT[64:128, 0:HT * 128],
                                      kT[0:64, 0:HT * 128])
                    for g in (2, 3):
                        ph2_group(g, ph1t)

                    # chunk 0 (single streams), half-1 transposes between
                    for h in range(G):
                        for _ in stream(h, 0, att, psc, ppv, pytu):
                            pass
                        if h < 2:
                            ph2_transpose(2 + h, ptr)
                    nc.sync.dma_start(kT[64:128, HT * 128:S],
                                      kT[0:64, HT * 128:S])
                    epilogue(0)

                # --- scope 2: remaining chunks, paired streams ---
                with (
                    tc.tile_pool(name="psc2", bufs=2, space="PSUM") as psc2,
                    tc.tile_pool(name="ppv2", bufs=2, space="PSUM") as ppv2,
                ):
                    for C in range(1, NCH):
                        for hp in range(0, G, 2):
                            ga = stream(hp, C, att, psc2, ppv2, pytu)
                            gb = stream(hp + 1, C, att, psc2, ppv2, pytu)
                            alive = [ga, gb]
                            while alive:
                                for g_ in list(alive):
                                    try:
                                        next(g_)
                                    except StopIteration:
                                        alive.remove(g_)
                        epilogue(C)

            # ---------------- phase 4: output projection -------------------
            with (
                tc.tile_pool(name="ob", bufs=3) as pob,
                tc.tile_pool(name="po", bufs=4, space="PSUM") as ppo,
            ):
                for t in range(T):
                    ob = pob.tile([128, E], F32)
                    for nh in range(2):
                        po = ppo.tile([128, 512], F32)
                        nc.tensor.matmul(
                            po[:], yT[0][:, t * 128:(t + 1) * 128],
                            wp_s[:, 0, nh * 512:(nh + 1) * 512],
                            start=True, stop=False)
                        nc.tensor.matmul(
                            po[:], yT[1][:, t * 128:(t + 1) * 128],
                            wp_s[:, 1, nh * 512:(nh + 1) * 512],
                            start=False, stop=True)
                        if (t + nh) % 2 == 0:
                            nc.vector.tensor_copy(
                                ob[:, nh * 512:(nh + 1) * 512], po[:])
                        else:
                            nc.scalar.copy(
                                ob[:, nh * 512:(nh + 1) * 512], po[:])
                    nc.sync.dma_start(
                        d_out[t * 128:(t + 1) * 128, :], ob[:])
    nc.compile()
    return nc


def _prep_inputs(x, ve, cos, sin, Wq, Wk, Wv, Wproj, Wgate):
    cosn = np.asarray(cos, np.float32).reshape(S, 32)
    sinn = np.asarray(sin, np.float32).reshape(S, 32)
    cos4 = np.empty((128, S), np.float32)
    sin4 = np.empty((128, S), np.float32)
    for t in range(T):
        cos4[:, t * 128:(t + 1) * 128] = np.tile(
            cosn[t * 128:(t + 1) * 128], (1, 4))
        sin4[:, t * 128:(t + 1) * 128] = np.tile(
            sinn[t * 128:(t + 1) * 128], (1, 4))
    cos4 = cos4.astype(NPBF)
    sin4 = sin4.astype(NPBF)

    Wq = np.asarray(Wq, np.float32)
    Wk = np.asarray(Wk, np.float32)
    Wv = np.asarray(Wv, np.float32)
    Wproj = np.asarray(Wproj, np.float32)
    Wgate = np.asarray(Wgate, np.float32)
    maps = []
    wl_ = int(getattr(_prep_inputs, '_wl', 1024))
    wr_ = min(int(getattr(_prep_inputs, '_wr', 0)), 0)
    maskt = _mask_tiles(wl_, wr_)
    for core in range(NCORES):
        b, g = core // 4, core % 4
        xT = np.ascontiguousarray(
            np.asarray(x[b], np.float32).T).astype(NPBF)
        wg = np.zeros((E, 1), np.float32)
        wg[:GC, 0] = Wgate[:, g]
        wqkv = np.ascontiguousarray(np.concatenate([
            Wq[:, g * G * D:(g + 1) * G * D],
            Wk[:, g * D:(g + 1) * D],
            Wv[:, g * D:(g + 1) * D],
            wg, np.zeros((E, 1), np.float32)], axis=1)).astype(NPBF)
        veg = 2.0 * np.asarray(ve[b][:, g * D:(g + 1) * D], np.float32)
        ve2 = np.ascontiguousarray(
            veg.reshape(T, 128, D).transpose(1, 0, 2).reshape(
                128, T * D)).astype(NPBF)
        wproj = np.ascontiguousarray(
            Wproj[g * G * D:(g + 1) * G * D, :]).astype(NPBF)
        maps.append({"xT": xT, "wqkv": wqkv, "cos4": cos4, "sin4": sin4,
                     "ve2": ve2, "wproj": wproj,
                     "ident": np.eye(128).astype(NPBF),
                     "masks": maskt,
                     "zero512": np.zeros((128, 512), NPBF)})
    return maps


def kernel(x, ve, cos, sin, Wq, Wk, Wv, Wproj, Wgate,
           window_left, window_right):
    global last_results
    wl, wr = int(window_left), int(window_right)
    key = (wl, wr)
    if key not in _cache:
        _cache[key] = _build(wl, wr)
    nc = _cache[key]
    _prep_inputs._wl, _prep_inputs._wr = wl, wr
    maps = _prep_inputs(x, ve, cos, sin, Wq, Wk, Wv, Wproj, Wgate)
    res = run_bass_kernel_spmd(
        nc, maps, core_ids=list(range(NCORES)),
        trace=bool(int(os.environ.get("KERNEL_TRACE", "0"))))
    last_results = res
    out = np.zeros((B, S, E), np.float32)
    for core in range(NCORES):
        out[core // 4] += res.results[core]["outp"]
    return out


# revision 19
# speedup vs baseline: 1.5117x; 1.1311x over previous
"""Trainium2 Bass kernel for nn_CausalSelfAttention_39213051412899.

Sliding-window causal GQA attention with value-embedding gate.
Sharding: 8 cores = batch(2) x kv-group(4).  Each core computes its batch's
4 query heads / 1 kv head and a row-parallel partial of the output
projection; the host sums the 4 partials per batch.

v2: bf16 matmul operands (FWL weight loads), software-pipelined phases so
the PE never idles past the HAM window, post-exp binary masks on GPSIMD,
k-RMS folded into the exp scale AP, single activation-table set
(ln/exp for rsqrt and sigmoid), per-chunk softmax epilogue.

Self-contained: only needs numpy + the concourse tree staged in the
container at /opt/trn_rl_repo (environment, not problem files).
"""

import os
import sys

import numpy as np

try:
    import concourse.bass as bass  # noqa: F401
except ImportError:  # pragma: no cover
    sys.path.insert(0, "/opt/trn_rl_repo")

import ml_dtypes

import concourse.bass as bass
import concourse.tile as tile
from concourse import bacc
from concourse import mybir
from concourse.bass_utils import run_bass_kernel_spmd

F32 = mybir.dt.float32
BF16 = mybir.dt.bfloat16
AF = mybir.ActivationFunctionType
ALU = mybir.AluOpType
NPBF = ml_dtypes.bfloat16

B, S, E = 2, 2048, 1024
H, KV, D = 16, 4, 64
G = H // KV          # 4 q heads per kv head (per core)
GC = 128             # gate channels
EPS = 1.1920929e-07
T = S // 128         # 16 s-tiles
QKVW = G * D + D + D + 2   # 386 packed cols (q|k|v|gate|pad)
GATE_COL = G * D + 2 * D   # 384
NCORES = 8
HT = T // 2          # tiles per half
QT = 4               # tiles per phase-2 group
NQ = T // QT         # 4 groups (2 per half)
VW = D + 2           # vaug row stride (v | ones | pad) - keeps 4B alignment

_cache = {}
last_results = None   # test harness reads exec_time_ns off this


def _block_range(ti, tj, lo_delta, hi_delta):
    """Active query-tile range for key-tile tj (None if empty)."""
    lo = max(ti[0], tj - hi_delta)
    hi = min(ti[1], tj - lo_delta)
    if lo > hi:
        return None
    return lo, hi


def _mask_table(wl, wr_eff):
    """Distinct 0/1 mask tiles needed, keyed (kind, base) -> index."""
    lo_delta = -((127 + wl) // 128)
    hi_delta = (127 + wr_eff) // 128
    keys = {}
    for dt_ in range(lo_delta, hi_delta + 1):   # dt_ = tj - tb
        bw = wl + 128 * dt_
        if -127 <= bw < 127:
            keys.setdefault(("w", bw), len(keys))
        bc = wr_eff - 128 * dt_
        if -127 <= bc < 127:
            keys.setdefault(("c", bc), len(keys))
    return keys


def _mask_tiles(wl, wr_eff):
    keys = _mask_table(wl, wr_eff)
    n = max(1, len(keys))
    m = np.ones((128, n * 128), np.float32)
    rj = np.arange(128)[:, None]
    ri = np.arange(128)[None, :]
    for (kind, base), i in keys.items():
        if kind == "w":
            bad = (rj - ri + base) < 0
        else:
            bad = (ri - rj + base) < 0
        m[:, i * 128:(i + 1) * 128] = np.where(bad, 0.0, 1.0)
    return m.astype(NPBF)


def _build(wl, wr):
    wr_eff = min(int(wr), 0)
    wl = int(wl)
    lo_delta = -((127 + wl) // 128)    # tj - ti >= lo_delta
    hi_delta = (127 + wr_eff) // 128   # tj - ti <= hi_delta  (0 when wr>=0)

    nc = bacc.Bacc(None, target_bir_lowering=False)
    d_xT = nc.declare_dram_parameter("xT", [E, S], BF16, isOutput=False)
    d_wqkv = nc.declare_dram_parameter("wqkv", [E, QKVW], BF16, isOutput=False)
    d_cos4 = nc.declare_dram_parameter("cos4", [128, S], BF16, isOutput=False)
    d_sin4 = nc.declare_dram_parameter("sin4", [128, S], BF16, isOutput=False)
    d_ve2 = nc.declare_dram_parameter("ve2", [128, T * D], BF16, isOutput=False)
    d_wproj = nc.declare_dram_parameter("wproj", [G * D, E], BF16,
                                        isOutput=False)
    d_ident = nc.declare_dram_parameter("ident", [128, 128], BF16,
                                        isOutput=False)
    d_zero = nc.declare_dram_parameter("zero512", [128, 512], BF16,
                                       isOutput=False)
    mask_idx = _mask_table(wl, wr_eff)
    nmask = max(1, len(mask_idx))
    d_masks = nc.declare_dram_parameter("masks", [128, nmask * 128], BF16,
                                        isOutput=False)
    d_out = nc.declare_dram_parameter("outp", [S, E], F32, isOutput=True)

    # chunking: widest chunk for which every chunk has a full-span tj
    def _has_full(cs_tiles):
        for C in range(T // cs_tiles):
            ti_ = (cs_tiles * C, cs_tiles * C + cs_tiles - 1)
            if not any(_block_range(ti_, tj, lo_delta, hi_delta) == ti_
                       for tj in range(T)):
                return False
        return True
    CST = 8 if _has_full(8) else 4       # chunk tiles
    CS = CST * 128
    NCH = T // CST
    pad_mode = not _has_full(CST)

    with tile.TileContext(nc) as tc:
        with tc.tile_pool(name="persist", bufs=1) as persist:
            qT = [persist.tile([128, S], BF16, tag=f"qT{i}", name=f"qT{i}")
                  for i in range(2)]                       # 2 heads per tile
            kT = persist.tile([128, S], BF16)              # k duplicated
            vaug = persist.tile([128, T, VW], BF16)        # v | ones | pad
            yT = [persist.tile([128, S], BF16, tag=f"yT{i}", name=f"yT{i}")
                  for i in range(2)]
            ident = persist.tile([128, 128], BF16)
            masks = persist.tile([128, nmask * 128], BF16)
            cos4 = persist.tile([128, S], BF16)
            sin4 = persist.tile([128, S], BF16)
            ve2 = persist.tile([128, T * D], BF16)
            wq_s = persist.tile([128, 8, QKVW], BF16)
            wp_s = persist.tile([128, 2, E], BF16)
            sigt = persist.tile([128, T], F32)             # sigmoid gate
            gtmp = persist.tile([128, T], F32)
            statq = persist.tile([128, 4 * T], F32)        # q sumsq (t,h)
            statk = persist.tile([128, T], F32)            # k sumsq
            rsq = persist.tile([128, 4 * T], BF16)         # 1/rms(q)
            rskD = persist.tile([128, T], F32)             # D^-.5/rms(k)
            Lt = persist.tile([64, CS], BF16)              # softmax denoms
            Linvb = persist.tile([64, CS], BF16)           # chunk C rows at 32C
            qkvs = [persist.tile([128, HT, QKVW], BF16, tag=f"qkv{i}",
                                 name=f"qkv{i}") for i in range(2)]
            epsq = persist.tile([128, 1], F32)
            epsk = persist.tile([128, 1], F32)
            nc.vector.memset(epsq[:], EPS)
            nc.vector.memset(epsk[:], float(EPS * D))

            # prologue DMAs: phase-2 constants from the scalar queue (idle
            # early), x tiles + weights from sync
            nc.scalar.dma_start(ident[:], d_ident[:, :])
            nc.scalar.dma_start(masks[:], d_masks[:, :])
            nc.scalar.dma_start(cos4[:], d_cos4[:, :])
            nc.scalar.dma_start(sin4[:], d_sin4[:, :])
            nc.scalar.dma_start(ve2[:], d_ve2[:, :])
            for kc in range(2):
                nc.scalar.dma_start(wp_s[:, kc, :],
                                    d_wproj[kc * 128:(kc + 1) * 128, :])
            nc.vector.memset(vaug[:, :, D:D + 1], 1.0)

            # ---------------- phase 1: qkv projections ---------------------
            with (
                tc.tile_pool(name="ph1x", bufs=2) as ph1x,
                tc.tile_pool(name="pq", bufs=3, space="PSUM") as pq,
            ):
                for hf in range(2):
                    xts = ph1x.tile([128, 8, HT * 128], BF16, tag="xts")
                    for c in range(8):
                        if hf == 0:
                            nc.sync.dma_start(wq_s[:, c, :],
                                              d_wqkv[c * 128:(c + 1) * 128, :])
                        if hf == 0:
                            hw = HT * 128 // 2
                            for sb in range(2):
                                nc.sync.dma_start(
                                    xts[:, c, sb * hw:(sb + 1) * hw],
                                    d_xT[c * 128:(c + 1) * 128,
                                         sb * hw:(sb + 1) * hw])
                        else:
                            nc.sync.dma_start(
                                xts[:, c, :],
                                d_xT[c * 128:(c + 1) * 128,
                                     hf * HT * 128:(hf + 1) * HT * 128])
                    qkv = qkvs[hf]
                    for t in range(HT):
                        ps = pq.tile([128, QKVW], F32)
                        for c in range(8):
                            nc.tensor.matmul(
                                ps[:], xts[:, c, t * 128:(t + 1) * 128],
                                wq_s[:, c, :],
                                start=(c == 0), stop=(c == 7))
                        nc.vector.tensor_copy(qkv[:, t, :], ps[:])

            # ---------------- phase 2 + 3 interleaved ----------------------
            rkq = {}

            def ph2_group(g, ph1t):
                """rope/rms/gate for tiles [4g, 4g+4) (no transposes)."""
                hf = g // 2
                t0 = g * QT                  # global tile base
                l0 = (g % 2) * QT            # local tile base within half
                qkv = qkvs[hf]
                qv = qkv[:, l0:l0 + QT, 0:G * D].rearrange(
                    "p t (h d) -> p t h d", h=G)
                kv_ = qkv[:, l0:l0 + QT, G * D:G * D + D].rearrange(
                    "p t (o d) -> p t o d", o=1)

                # sum-of-squares from pre-rope q/k (rope preserves norms)
                sqq = ph1t.tile([128, QT, G, D], BF16, tag="sqq")
                sqk = ph1t.tile([128, QT, 1, D], BF16, tag="sqk")
                nc.vector.tensor_mul(sqq[:], qv, qv)
                nc.vector.tensor_mul(sqk[:], kv_, kv_)
                nc.vector.tensor_reduce(
                    op=ALU.add,
                    out=statq[:, t0 * 4:(t0 + QT) * 4].rearrange(
                        "p (t h) -> p t h", h=G),
                    in_=sqq[:], axis=mybir.AxisListType.X)
                nc.vector.tensor_reduce(
                    op=ALU.add,
                    out=statk[:, t0:t0 + QT].rearrange(
                        "p (t h) -> p t h", h=1),
                    in_=sqk[:], axis=mybir.AxisListType.X)
                # 1/sqrt via Sqrt + DVE reciprocal
                qs = slice(t0 * 4, (t0 + QT) * 4)
                nc.scalar.activation(statq[:, qs], statq[:, qs], AF.Sqrt,
                                     bias=epsq[:], scale=1.0 / D)
                with nc.allow_low_precision(
                        reason="bf16 rms scale, 0.4% ok for 2e-2 budget"):
                    nc.vector.reciprocal(rsq[:, qs], statq[:, qs])
                ks = slice(t0, t0 + QT)
                nc.scalar.activation(statk[:, ks], statk[:, ks], AF.Sqrt,
                                     bias=epsk[:], scale=1.0)
                nc.vector.reciprocal(rskD[:, ks], statk[:, ks])

                # rope
                cosq = cos4[:, t0 * 128:(t0 + QT) * 128].rearrange(
                    "p (t h f) -> p t h f", h=G, f=32)
                sinq = sin4[:, t0 * 128:(t0 + QT) * 128].rearrange(
                    "p (t h f) -> p t h f", h=G, f=32)
                rq = ph1t.tile([128, QT, G, D], BF16, tag="rq")
                rk = ph1t.tile([128, QT, 1, D], BF16, tag="rk")
                rkq[g] = rk
                tmp = ph1t.tile([128, QT, G, 32], BF16, tag="tmp")
                tmpk = ph1t.tile([128, QT, 1, 32], BF16, tag="tmpk")
                for (src, dst, cs, sn, tm) in (
                        (qv, rq, cosq, sinq, tmp),
                        (kv_, rk, cosq[:, :, 0:1, :], sinq[:, :, 0:1, :],
                         tmpk)):
                    x1 = src[:, :, :, 0:32]
                    x2 = src[:, :, :, 32:64]
                    nc.vector.tensor_mul(dst[:, :, :, 0:32], x1, cs)
                    nc.vector.tensor_mul(tm[:], x2, sn)
                    nc.vector.tensor_add(
                        dst[:, :, :, 0:32], dst[:, :, :, 0:32], tm[:])
                    nc.vector.tensor_mul(dst[:, :, :, 32:64], x2, cs)
                    nc.vector.tensor_mul(tm[:], x1, sn)
                    nc.vector.tensor_sub(
                        dst[:, :, :, 32:64], dst[:, :, :, 32:64], tm[:])

                # normalize q back into qkv (k stays raw; rms in exp scale)
                rsb = rsq[:, qs].rearrange("p (t h o) -> p t h o", h=G, o=1)
                nc.vector.tensor_mul(qv, rq[:], rsb.to_broadcast(
                    [128, QT, G, D]))

                # v + sig * ve2  (ve2 pre-scaled by 2 on host)
                tmpv = ph1t.tile([128, QT, D], BF16, tag="tmpv")
                for t in range(QT):
                    tt = t0 + t
                    nc.vector.tensor_scalar_mul(
                        tmpv[:, t, :], ve2[:, tt * D:(tt + 1) * D],
                        sigt[:, tt:tt + 1])
                nc.vector.tensor_add(
                    vaug[:, t0:t0 + QT, 0:D],
                    qkv[:, l0:l0 + QT, G * D + D:G * D + 2 * D], tmpv[:])

            def ph2_rs(hf):
                b0 = hf * 40
                nc.scalar.activation(stat[:, b0:b0 + 32], stat[:, b0:b0 + 32],
                                     AF.Sqrt, bias=epsq[:], scale=1.0 / D)
                nc.scalar.activation(stat[:, b0 + 32:b0 + 40],
                                     stat[:, b0 + 32:b0 + 40],
                                     AF.Sqrt, bias=epsk[:], scale=1.0)
                with nc.allow_low_precision(
                        reason="bf16 rms scales, 0.4% ok for 2e-2 budget"):
                    nc.vector.reciprocal(rs[:, b0:b0 + 40],
                                         stat[:, b0:b0 + 40])

            def ph2_norm(g):
                hf = g // 2
                lt0 = (g % 2) * QT
                b0 = hf * 40
                l0 = lt0
                qkv = qkvs[hf]
                qv = qkv[:, l0:l0 + QT, 0:G * D].rearrange(
                    "p t (h d) -> p t h d", h=G)
                kv_ = qkv[:, l0:l0 + QT, G * D:G * D + D].rearrange(
                    "p t (o d) -> p t o d", o=1)
                qs = slice(b0 + lt0 * 4, b0 + lt0 * 4 + 4 * QT)
                ks = slice(b0 + 32 + lt0, b0 + 32 + lt0 + QT)
                rsb = rs[:, qs].rearrange("p (t h o) -> p t h o", h=G, o=1)
                nc.vector.tensor_mul(qv, rqs[g][:], rsb.to_broadcast(
                    [128, QT, G, D]))
                rkb = rs[:, ks].rearrange("p (t h o) -> p t h o", h=1, o=1)
                nc.vector.tensor_mul(kv_, rks[g][:], rkb.to_broadcast(
                    [128, QT, 1, D]))

            def ph2_transpose(g, ptr):
                hf = g // 2
                t0 = g * QT
                l0 = (g % 2) * QT
                qkv = qkvs[hf]
                for t in range(QT):
                    tt = t0 + t
                    for bk in range(2):
                        tp = ptr.tile([128, 128], BF16, tag="tp", name="tp")
                        nc.tensor.transpose(
                            tp[:], qkv[:, l0 + t, bk * 128:(bk + 1) * 128],
                            ident[:])
                        nc.vector.tensor_copy(
                            qT[bk][:, tt * 128:(tt + 1) * 128], tp[:])
                    tp = ptr.tile([128, 128], BF16, tag="tp", name="tp")
                    nc.tensor.transpose(
                        tp[0:64, :],
                        rkq[g][:, t, 0, :],
                        ident[:])
                    nc.vector.tensor_copy(
                        kT[0:64, tt * 128:(tt + 1) * 128], tp[0:64, :])

            def gate_sig(hf):
                qkv = qkvs[hf]
                t0 = hf * HT
                nc.scalar.activation(
                    sigt[:, t0:t0 + HT],
                    qkv[:, :, GATE_COL:GATE_COL + 1].rearrange(
                        "p t o -> p (t o)"),
                    AF.Sigmoid)

            ytus = {}

            def stream(h, C, att, psc, ppv, pytu):
                """One (head, chunk) attention stream."""
                rh = slice((h % 2) * 64, (h % 2) * 64 + 64)
                qTh = qT[h // 2]
                c0 = CST * C
                ti = (c0, c0 + CST - 1)
                tjs = [tj for tj in
                       range(max(0, c0 + lo_delta),
                             min(T - 1, c0 + CST - 1 + hi_delta) + 1)
                       if _block_range(ti, tj, lo_delta, hi_delta)]
                full = [tj for tj in tjs
                        if _block_range(ti, tj, lo_delta, hi_delta) == ti]
                if pad_mode:
                    order = tjs
                else:
                    ftj = full[-1]
                    order = [ftj] + [tj for tj in tjs if tj != ftj]

                yTa = ppv.tile([65, CS], F32, tag="yTa", name="yTa")
                half_started = [False] * (CST // 4)
                half_last = {}
                for i, tj in enumerate(order):
                    lo_, hi_ = _block_range(ti, tj, lo_delta, hi_delta)
                    o_, n_ = ((0, CS) if (pad_mode and i == 0) else
                              ((lo_ - c0) * 128, (hi_ - lo_ + 1) * 128))
                    for hx in range(CST // 4):
                        if o_ < (hx + 1) * 512 and o_ + n_ > hx * 512:
                            half_last[hx] = i
                pend = []

                def emit_pv(rec):
                    i, tj, pt, off, n = rec
                    for hx in range(CST // 4):
                        h0_ = hx * 512
                        s0 = max(off, h0_)
                        s1 = min(off + n, h0_ + 512)
                        if s0 >= s1:
                            continue
                        first = not half_started[hx]
                        half_started[hx] = True
                        nc.tensor.matmul(
                            yTa[:, s0:s1], vaug[:, tj, 0:D + 1],
                            pt[:, s0:s1],
                            start=first, stop=(half_last[hx] == i))

                for i, tj in enumerate(order):
                    alo, ahi = _block_range(ti, tj, lo_delta, hi_delta)
                    aoff = (alo - c0) * 128
                    an = (ahi - alo + 1) * 128
                    if pad_mode and i == 0:
                        off, n = 0, CS
                    else:
                        off, n = aoff, an
                    sc = psc.tile([128, CS], F32, tag="sc", name="sc")
                    p0 = aoff
                    while p0 < aoff + an:
                        p1 = min((p0 // 512 + 1) * 512, aoff + an)
                        nc.tensor.matmul(
                            sc[:, p0:p1],
                            kT[rh, tj * 128:(tj + 1) * 128],
                            qTh[rh, C * CS + p0:C * CS + p1],
                            start=True, stop=True)
                        p0 = p1
                    yield
                    pt = att.tile([128, CS], BF16, tag="pt", name="pt")
                    if pad_mode:
                        nc.sync.dma_start(
                            pt[:], d_zero[:, :].to_broadcast([128, CS]))
                    nc.scalar.activation(
                        pt[:, aoff:aoff + an], sc[:, aoff:aoff + an],
                        AF.Exp, scale=rskD[:, tj:tj + 1])
                    # post-exp 0/1 masks on gpsimd
                    for tb in range(alo, ahi + 1):
                        bo = (tb - c0) * 128
                        for kind, base in (("w", wl - 128 * (tb - tj)),
                                           ("c", wr_eff + 128 * (tb - tj))):
                            if -127 <= base < 127:
                                mi = mask_idx[(kind, base)]
                                nc.vector.tensor_mul(
                                    pt[:, bo:bo + 128],
                                    pt[:, bo:bo + 128],
                                    masks[:, mi * 128:(mi + 1) * 128])
                    pend.append((i, tj, pt, off, n))
                    if len(pend) > 1:
                        emit_pv(pend.pop(0))
                    yield
                emit_pv(pend.pop(0))

                ytu = pytu.tile([65, CS], BF16, tag="ytu", name="ytu")
                nc.vector.tensor_copy(ytu[:], yTa[:])
                rr = C * 32 + h
                nc.sync.dma_start(Lt[rr:rr + 1, 0:CS], ytu[64:65, :])
                ytus[(h, C)] = ytu

            with (
                tc.tile_pool(name="ph1t", bufs=2) as ph1t,
                tc.tile_pool(name="att", bufs=6) as att,
                tc.tile_pool(name="ytu", bufs=G * NCH) as pytu,
                tc.tile_pool(name="lbp", bufs=4) as plb,
                tc.tile_pool(name="ytn", bufs=2) as pytn,
                tc.tile_pool(name="dsc", bufs=1, space="DRAM") as dsc,
            ):
                d_linv = dsc.tile([G * NCH, CS], BF16)

                def epilogue(C):
                    a0 = C * 32
                    with nc.allow_low_precision(
                            reason="1/L in bf16; 0.4% on softmax denom is "
                                   "fine for the 2e-2 budget"):
                        nc.vector.reciprocal(Linvb[a0:a0 + G, 0:CS],
                                             Lt[a0:a0 + G, 0:CS])
                    nc.sync.dma_start(d_linv[C * G:(C + 1) * G, :],
                                      Linvb[a0:a0 + G, 0:CS])
                    ccols = slice(C * CS, (C + 1) * CS)
                    for h in range(G):
                        rr = C * G + h
                        ytu = ytus[(h, C)]
                        lb = plb.tile([64, CS], BF16)
                        nc.sync.dma_start(
                            lb[:],
                            d_linv[rr:rr + 1, :].to_broadcast([64, CS]))
                        if h % 2 == 0:
                            nc.vector.tensor_mul(
                                yT[h // 2][0:64, ccols], ytu[0:64, :], lb[:])
                        else:
                            ytn = pytn.tile([64, CS], BF16)
                            nc.vector.tensor_mul(ytn[:], ytu[0:64, :], lb[:])
                            nc.sync.dma_start(yT[h // 2][64:128, ccols],
                                              ytn[:])

                # --- scope 1: phase 2 + chunk 0 (single streams) ---
                with (
                    tc.tile_pool(name="ptr", bufs=2, space="PSUM") as ptr,
                    tc.tile_pool(name="psc", bufs=2, space="PSUM") as psc,
                    tc.tile_pool(name="ppv", bufs=1, space="PSUM") as ppv,
                ):
                    gate_sig(0)
                    for g in (0, 1):
                        ph2_group(g, ph1t)
                    ph2_rs(0)
                    gate_sig(1)
                    for g in (0, 1):
                        ph2_norm(g)
                        ph2_transpose(g, ptr)
                    nc.sync.dma_start(kT[64:128, 0:HT * 128],
                                      kT[0:64, 0:HT * 128])
                    for g in (2, 3):
                        ph2_group(g, ph1t)

                    # chunk 0 (single streams); half-1 rs/norm/transposes
                    # interleaved so the scalar queue stays load-batched
                    for h in range(G):
                        for _ in stream(h, 0, att, psc, ppv, pytu):
                            pass
                        if h == 0:
                            ph2_rs(1)
                            ph2_norm(2)
                            ph2_norm(3)
                        elif h < 3:
                            ph2_transpose(1 + h, ptr)
                    nc.sync.dma_start(kT[64:128, HT * 128:S],
                                      kT[0:64, HT * 128:S])
                    epilogue(0)

                # --- scope 2: remaining chunks, paired streams ---
                with (
                    tc.tile_pool(name="psc2", bufs=2, space="PSUM") as psc2,
                    tc.tile_pool(name="ppv2", bufs=2, space="PSUM") as ppv2,
                ):
                    for C in range(1, NCH):
                        for hp in range(0, G, 2):
                            ga = stream(hp, C, att, psc2, ppv2, pytu)
                            gb = stream(hp + 1, C, att, psc2, ppv2, pytu)
                            alive = [ga, gb]
                            while alive:
                                for g_ in list(alive):
                                    try:
                                        next(g_)
                                    except StopIteration:
                                        alive.remove(g_)
                        epilogue(C)

            # ---------------- phase 4: output projection -------------------
            with (
                tc.tile_pool(name="ob", bufs=3) as pob,
                tc.tile_pool(name="po", bufs=4, space="PSUM") as ppo,
            ):
                for t in range(T):
                    ob = pob.tile([128, E], F32)
                    for nh in range(2):
                        po = ppo.tile([128, 512], F32)
                        nc.tensor.matmul(
                            po[:], yT[0][:, t * 128:(t + 1) * 128],
                            wp_s[:, 0, nh * 512:(nh + 1) * 512],
                            start=True, stop=False)
                        nc.tensor.matmul(
                            po[:], yT[1][:, t * 128:(t + 1) * 128],
                            wp_s[:, 1, nh * 512:(nh + 1) * 512],
                            start=False, stop=True)
                        if (t + nh) % 2 == 0:
                            nc.vector.tensor_copy(
                                ob[:, nh * 512:(nh + 1) * 512], po[:])
                        else:
                            nc.scalar.copy(
                                ob[:, nh * 512:(nh + 1) * 512], po[:])
                    nc.sync.dma_start(
                        d_out[t * 128:(t + 1) * 128, :], ob[:])
    nc.compile()
    return nc


def _prep_inputs(x, ve, cos, sin, Wq, Wk, Wv, Wproj, Wgate):
    cosn = np.asarray(cos, np.float32).reshape(S, 32)
    sinn = np.asarray(sin, np.float32).reshape(S, 32)
    cos4 = np.empty((128, S), np.float32)
    sin4 = np.empty((128, S), np.float32)
    for t in range(T):
        cos4[:, t * 128:(t + 1) * 128] = np.tile(
            cosn[t * 128:(t + 1) * 128], (1, 4))
        sin4[:, t * 128:(t + 1) * 128] = np.tile(
            sinn[t * 128:(t + 1) * 128], (1, 4))
    cos4 = cos4.astype(NPBF)
    sin4 = sin4.astype(NPBF)

    Wq = np.asarray(Wq, np.float32)
    Wk = np.asarray(Wk, np.float32)
    Wv = np.asarray(Wv, np.float32)
    Wproj = np.asarray(Wproj, np.float32)
    Wgate = np.asarray(Wgate, np.float32)
    maps = []
    wl_ = int(getattr(_prep_inputs, '_wl', 1024))
    wr_ = min(int(getattr(_prep_inputs, '_wr', 0)), 0)
    maskt = _mask_tiles(wl_, wr_)
    for core in range(NCORES):
        b, g = core // 4, core % 4
        xT = np.ascontiguousarray(
            np.asarray(x[b], np.float32).T).astype(NPBF)
        wg = np.zeros((E, 1), np.float32)
        wg[:GC, 0] = Wgate[:, g]
        wqkv = np.ascontiguousarray(np.concatenate([
            Wq[:, g * G * D:(g + 1) * G * D],
            Wk[:, g * D:(g + 1) * D],
            Wv[:, g * D:(g + 1) * D],
            wg, np.zeros((E, 1), np.float32)], axis=1)).astype(NPBF)
        veg = 2.0 * np.asarray(ve[b][:, g * D:(g + 1) * D], np.float32)
        ve2 = np.ascontiguousarray(
            veg.reshape(T, 128, D).transpose(1, 0, 2).reshape(
                128, T * D)).astype(NPBF)
        wproj = np.ascontiguousarray(
            Wproj[g * G * D:(g + 1) * G * D, :]).astype(NPBF)
        maps.append({"xT": xT, "wqkv": wqkv, "cos4": cos4, "sin4": sin4,
                     "ve2": ve2, "wproj": wproj,
                     "ident": np.eye(128).astype(NPBF),
                     "masks": maskt,
                     "zero512": np.zeros((128, 512), NPBF)})
    return maps


def kernel(x, ve, cos, sin, Wq, Wk, Wv, Wproj, Wgate,
           window_left, window_right):
    global last_results
    wl, wr = int(window_left), int(window_right)
    key = (wl, wr)
    if key not in _cache:
        _cache[key] = _build(wl, wr)
    nc = _cache[key]
    _prep_inputs._wl, _prep_inputs._wr = wl, wr
    maps = _prep_inputs(x, ve, cos, sin, Wq, Wk, Wv, Wproj, Wgate)
    res = run_bass_kernel_spmd(
        nc, maps, core_ids=list(range(NCORES)),
        trace=bool(int(os.environ.get("KERNEL_TRACE", "0"))))
    last_results = res
    out = np.zeros((B, S, E), np.float32)
    for core in range(NCORES):
        out[core // 4] += res.results[core]["outp"]
    return out
